# revision 61
# baseline (speedup 1.0000x reference)
"""3-layer GCN on 8 Trainium2 NeuronCores (Bass/Tile SPMD kernel).

Strategy (sharding_hint: shard nodes + edges by destination, replicate
weights, all-gather activations between layers):

  - Nodes are split into 8 contiguous blocks (padded to a multiple of 128
    rows per core).  Core c owns destination block c.
  - Per layer, using linearity of GCN aggregation:
        out_i = [sum_{j->i} dinv_i dinv_j p_j + dinv_i^2 p_i] @ W + b
    with p = previous activations.  We store ps = dinv * p ("scaled"
    activations) so every message (including the self loop, added as an
    explicit edge) has unit coefficient:
        z_i = dinv_i * segment_sum(ps[src])       (over edges + self edges)
        out_i = z_i @ W + b ; next ps = dinv * relu(out)
  - Each core DMA-gathers ps[src] rows (512B each) for its edges from a
    replicated full-activation DRAM buffer, reduces them into per-256-dst
    "window pair" PSUM tiles with one-hot matmuls (one-hots built by DVE
    is_equal against an iota row), applies dinv, multiplies by W (PE),
    bias+ReLU (ACT), rescales, and writes its 1/8 output block.
  - An AllGather (ncfw collective) replicates the per-core ps blocks
    between layers.  Matmul operands use float32r (TF32-like, ~1e-4 rel).

Edges are sorted by (dst core, dst window pair, src chunk); each
(pair, chunk) cell is padded to whole 128-token groups with a group count
equalized across cores so a single SPMD program serves all 8 cores.
Gather indices are int16 (chunk-relative, chunks of <=32768 rows).
"""

import sys

if "/opt/trn_rl_repo" not in sys.path:
    sys.path.insert(0, "/opt/trn_rl_repo")

import numpy as np

import concourse.bacc as bacc
import concourse.mybir as mybir
import concourse.tile as tile
from concourse import bass_utils

F32 = mybir.dt.float32
F32R = mybir.dt.float32r
F16 = mybir.dt.float16
I16 = mybir.dt.int16

NCORES = 8
D = 128
USE_F32R = True      # float32r (TF32-like) matmul operands; False = exact fp32
USE_F16_MSG = True   # fp16 gathered messages + one-hot S (halves gather bytes)
import os as _os

BATCH_ISEQ = _os.environ.get("BATCH_ISEQ", "1") == "1"
SINGLE_PACKET = _os.environ.get("SINGLE_PACKET", "0") == "1"
BALANCE = _os.environ.get("BALANCE", "1") == "1"
SELF_EDGES = _os.environ.get("SELF_EDGES", "0") == "1"
NQ = 4               # SWDGE queues for gather desc-gen parallelism
PAIRW = 256          # dst window width (S matrix / PSUM free size)
WPP = PAIRW // 128   # 128-row windows per dst bin
RP = 2               # dst bins per gather range
NCHUNKS = 4          # gather source chunks = collective quarters


def _assign_balanced(dst, n_nodes, NPAIR):
    """LPT greedy: assign dst rows to (core, pair) bins equalizing total
    token count per bin (in-degree + self loop), capacity PAIRW rows.

    Per-chunk counts then vary only by the multinomial split of each
    bin's sources over the 4 chunks (sigma ~ sqrt(total)/2), vs the
    ~4x larger variance of blocked assignment."""
    import heapq

    NBINS = NCORES * NPAIR
    indeg = np.bincount(dst, minlength=n_nodes).astype(np.int64)
    if SELF_EDGES:
        indeg += 1
    order = np.argsort(-indeg, kind="stable")
    cap = np.full(NBINS, PAIRW, dtype=np.int64)
    assign = np.empty(n_nodes, dtype=np.int64)
    heap = [(0, b) for b in range(NBINS)]
    heapq.heapify(heap)
    for i in order:
        while True:
            load, b = heapq.heappop(heap)
            if cap[b] > 0:
                break
        assign[i] = b
        cap[b] -= 1
        if cap[b] > 0:
            heapq.heappush(heap, (load + int(indeg[i]), b))
    return assign


def _preprocess(edge_index, n_nodes):
    """Host-side integer preprocessing: degrees, edge partition, padding.

    Returns a dict with the static program structure (identical across
    cores) and per-core gather/one-hot metadata arrays.
    """
    src = np.asarray(edge_index[0], dtype=np.int64)
    dst = np.asarray(edge_index[1], dtype=np.int64)

    nb_real = -(-n_nodes // NCORES)              # real rows per core
    NB = -(-nb_real // 128) * 128                # padded rows per core
    NPAD = NB * NCORES
    NW = NB // 128                               # windows per core
    NPAIR = -(-NW // WPP)                        # dst bins per core
    nchunks = NCHUNKS
    # quarter q covers qwin[q] windows per core; chunk q of the gather
    # source = the 8 cores' quarter-q sub-blocks (one AllGather each)
    qbase = (NW // (nchunks * RP)) * RP
    qwin = [qbase] * (nchunks - 1) + [NW - qbase * (nchunks - 1)]
    wstart = np.concatenate([[0], np.cumsum(qwin)]).astype(np.int64)
    csize = [q * 128 * NCORES for q in qwin]     # rows per chunk
    chunk_base = np.concatenate([[0], np.cumsum(csize)]).astype(np.int64)
    assert max(csize) < 32768, csize
    quarter_of_w = np.repeat(np.arange(nchunks), qwin)

    deg = np.ones(n_nodes, dtype=np.float64)
    np.add.at(deg, dst, 1.0)                     # bincount, +1 self loop
    deg = deg.astype(np.float32)

    allnodes = np.arange(n_nodes, dtype=np.int64)

    if BALANCE:
        bin_of = _assign_balanced(dst, n_nodes, NPAIR)
        order_r = np.argsort(bin_of, kind="stable")
        poff_node = np.empty(n_nodes, dtype=np.int64)
        starts = np.zeros(NCORES * NPAIR + 1, dtype=np.int64)
        np.cumsum(np.bincount(bin_of, minlength=NCORES * NPAIR),
                  out=starts[1:])
        poff_node[order_r] = np.arange(n_nodes) - starts[bin_of[order_r]]
    else:
        bin_of = ((allnodes // nb_real) * NPAIR
                  + (allnodes % nb_real) // PAIRW)
        poff_node = (allnodes % nb_real) % PAIRW

    # local row -> global padded row (quarter-major across cores)
    qwin_a = np.asarray(qwin, dtype=np.int64)
    loc_r = np.arange(NB, dtype=np.int64)
    wq = quarter_of_w[loc_r // 128]
    glob_of_local = np.empty((NCORES, NB), dtype=np.int64)
    for c in range(NCORES):
        glob_of_local[c] = (chunk_base[wq] + c * (qwin_a[wq] * 128)
                            + (loc_r - wstart[wq] * 128))
    loc_node = (bin_of % NPAIR) * PAIRW + poff_node
    gp_node = glob_of_local[bin_of // NPAIR, loc_node]

    # self-loop terms are added on-chip (PSUM init transposes), not as
    # edges — unless SELF_EDGES (debug fallback to explicit self tokens)
    if SELF_EDGES:
        s_all = np.concatenate([src, allnodes])
        d_all = np.concatenate([dst, allnodes])
    else:
        s_all = src
        d_all = dst

    core = bin_of[d_all] // NPAIR
    pair = bin_of[d_all] % NPAIR
    poff = poff_node[d_all]                      # offset within bin
    sgp = gp_node[s_all]
    chunk = np.searchsorted(chunk_base, sgp, side="right") - 1
    sidx = (sgp - chunk_base[chunk]).astype(np.int64)

    # sort by (core, pair, chunk), then src within each cell (HBM locality)
    key = ((core * NPAIR) + pair) * nchunks + chunk
    order = np.lexsort((sidx, key))
    key_s = key[order]
    sidx_s = sidx[order]
    poff_s = poff[order]

    ncells = NPAIR * nchunks
    # per-core per-cell counts
    counts = np.zeros((NCORES, ncells), dtype=np.int64)
    uk, uc = np.unique(key_s, return_counts=True)
    counts.reshape(-1)[uk] = uc
    gcell = (-(-counts // 128)).max(axis=0)      # equalized group counts
    gcell = gcell.reshape(NPAIR, nchunks)        # [pair, chunk]

    gtot = int(gcell.sum())
    # stream layout: for p in pairs: for k in chunks: gcell[p,k] groups
    cell_goff = np.zeros((NPAIR, nchunks), dtype=np.int64)
    g = 0
    for p in range(NPAIR):
        for k in range(nchunks):
            cell_goff[p, k] = g
            g += gcell[p, k]

    # per-core padded token arrays in stream order (token-major flat)
    dstw_flat = np.full((NCORES, gtot * 128), -1.0, dtype=np.float32)

    # chunk stream group offsets (within each chunk's gather stream)
    chunk_goff = np.zeros((NPAIR, nchunks), dtype=np.int64)
    acc = np.zeros(nchunks, dtype=np.int64)
    for p in range(NPAIR):
        for k in range(nchunks):
            chunk_goff[p, k] = acc[k]
            acc[k] += gcell[p, k]
    gchunk = acc                                  # groups per chunk stream

    idx_streams = [
        np.zeros((NCORES, int(gchunk[k]) * 128), dtype=np.int16)
        for k in range(nchunks)
    ]

    cell_starts = np.zeros(NCORES * ncells + 1, dtype=np.int64)
    np.cumsum(counts.reshape(-1), out=cell_starts[1:])
    for c in range(NCORES):
        for p in range(NPAIR):
            for k in range(nchunks):
                cell = (c * NPAIR + p) * nchunks + k
                t0, t1 = cell_starts[cell], cell_starts[cell + 1]
                n = t1 - t0
                gk0 = chunk_goff[p, k] * 128
                idx_streams[k][c, gk0 : gk0 + n] = sidx_s[t0:t1]
                g0 = cell_goff[p, k]
                dstw_flat[c, g0 * 128 : g0 * 128 + n] = poff_s[t0:t1]
                # pads keep idx 0 / dstw -1

    # dstw: token t of group g -> [t%128, g]
    dstw = np.ascontiguousarray(
        dstw_flat.reshape(NCORES, gtot, 128).transpose(0, 2, 1)
    )

    # wrap indices: token i -> [i%16, i//16], replicated to 128 partitions
    idx_wrapped = []
    for k in range(nchunks):
        st = idx_streams[k]
        cols = st.shape[1] // 16
        w = st.reshape(NCORES, cols, 16).transpose(0, 2, 1)  # [C,16,cols]
        idx_wrapped.append(np.tile(w, (1, 8, 1)).copy())     # [C,128,cols]

    # degree layouts
    deg_pad = np.ones(NPAD, dtype=np.float32)
    deg_pad[gp_node] = deg
    deg_w = np.empty((NCORES, 128, NW), dtype=np.float32)    # wrapped
    deg_r = np.empty((NCORES, 1, NB), dtype=np.float32)      # row
    for c in range(NCORES):
        blk = deg_pad[glob_of_local[c]]
        deg_w[c] = blk.reshape(NW, 128).T
        deg_r[c, 0] = blk

    return dict(
        NB=NB, NPAD=NPAD, NW=NW, NPAIR=NPAIR, nchunks=nchunks,
        nb_real=nb_real, gcell=gcell, gtot=gtot, gchunk=gchunk,
        cell_goff=cell_goff, chunk_goff=chunk_goff,
        idx_wrapped=idx_wrapped, dstw=dstw, deg_w=deg_w, deg_r=deg_r,
        gp_node=gp_node, glob_of_local=glob_of_local,
        csize=csize, wstart=wstart,
    )


def _build(meta):
    NB, NPAD, NW, NPAIR = meta["NB"], meta["NPAD"], meta["NW"], meta["NPAIR"]
    nchunks, gcell, gtot = meta["nchunks"], meta["gcell"], meta["gtot"]
    gchunk, cell_goff, chunk_goff = (
        meta["gchunk"], meta["cell_goff"], meta["chunk_goff"],
    )

    DT_R = F32R if USE_F32R else F32
    DT_M = F16 if USE_F16_MSG else DT_R     # message/S dtype

    ranges_l = [list(range(r, min(r + RP, NPAIR)))
                for r in range(0, NPAIR, RP)]
    NGRMAX = max(int(gcell[rp, :].sum()) for rp in ranges_l)

    nc = bacc.Bacc(None, target_bir_lowering=False, num_devices=NCORES,
                   num_swdge_queues=NQ)

    x_ext = nc.dram_tensor("x", [NB, D], F32, kind="ExternalInput")
    degw_ext = nc.dram_tensor("degw", [128, NW], F32, kind="ExternalInput")
    iota_ext = nc.dram_tensor("iota", [128, PAIRW],
                              F16 if USE_F16_MSG else F32,
                              kind="ExternalInput")
    ident_ext = nc.dram_tensor("ident", [128, 128], F32, kind="ExternalInput")
    ones_ext = nc.dram_tensor("ones1", [1, 128], F32, kind="ExternalInput")
    w_ext = [
        nc.dram_tensor(f"w{l}", [D, D], F32, kind="ExternalInput")
        for l in range(3)
    ]
    b_ext = [
        nc.dram_tensor(f"b{l}", [128, 1], F32, kind="ExternalInput")
        for l in range(3)
    ]
    idx_ext = [
        nc.dram_tensor(f"idx{k}", [128, int(gchunk[k]) * 8], I16,
                       kind="ExternalInput")
        for k in range(nchunks)
    ]
    dstw_ext = nc.dram_tensor("dstw", [128, gtot],
                              F16 if (USE_F16_MSG and BATCH_ISEQ) else F32,
                              kind="ExternalInput")
    out_ext = nc.dram_tensor("out", [NB, D], F32, kind="ExternalOutput")

    csize, wstart = meta["csize"], meta["wstart"]
    ps_loc = nc.dram_tensor("ps_loc", [NB, D], DT_M)
    ps_full = [
        nc.dram_tensor(f"ps_full{k}", [csize[k], D], DT_M,
                       addr_space="Shared")
        for k in range(nchunks)
    ]

    def allgather_ps():
        # one AllGather per quarter so next-layer gathers on chunk k
        # unblock as soon as quarter k lands (not the whole layer)
        for k in range(nchunks):
            lo, hi = int(wstart[k]) * 128, int(wstart[k + 1]) * 128
            nc.gpsimd.collective_compute(
                "AllGather", mybir.AluOpType.bypass,
                replica_groups=[list(range(NCORES))],
                ins=[ps_loc[lo:hi, :].opt()],
                outs=[ps_full[k].ap().opt()],
            )

    QROT = [0]

    # gather ranges: RP pairs each
    ranges = [list(range(r, min(r + RP, NPAIR))) for r in range(0, NPAIR, RP)]

    with tile.TileContext(nc) as tc:
        with (
            tc.tile_pool(name="const", bufs=1) as cpool,
            tc.tile_pool(name="msg", bufs=2) as mpool,
            tc.tile_pool(name="idxp", bufs=2) as ipool,
            tc.tile_pool(name="sbld", bufs=2) as spool,
            tc.tile_pool(name="work", bufs=3) as wpool,
            tc.tile_pool(name="outp", bufs=4) as opool,
            tc.tile_pool(name="pz", bufs=2, space="PSUM") as pzpool,
            tc.tile_pool(name="pt", bufs=2, space="PSUM") as ptpool,
            tc.tile_pool(name="ph", bufs=2, space="PSUM") as phpool,
        ):
            # ---- constants ----
            iota_sb = cpool.tile([128, PAIRW], F16 if USE_F16_MSG else F32)
            nc.sync.dma_start(out=iota_sb[:], in_=iota_ext[:, :])
            ident_sb = cpool.tile([128, 128], F32)
            nc.sync.dma_start(out=ident_sb[:], in_=ident_ext[:, :])
            ident16 = cpool.tile([128, 128], DT_M, tag="ident16")
            nc.vector.tensor_copy(ident16[:], ident_sb[:])
            ones_sb = cpool.tile([1, 128], F32)
            nc.sync.dma_start(out=ones_sb[:], in_=ones_ext[:, :])
            w_sb = []
            for l in range(3):
                wt = cpool.tile([D, D], F32, tag=f"wraw{l}")
                nc.sync.dma_start(out=wt[:], in_=w_ext[l][:, :])
                if USE_F32R:
                    wr = cpool.tile([D, D], F32R, tag=f"w{l}")
                    nc.vector.tensor_copy(wr[:], wt[:])
                    w_sb.append(wr)
                else:
                    w_sb.append(wt)
            b_sb = []
            for l in range(3):
                bt = cpool.tile([128, 1], F32, tag=f"b{l}")
                nc.sync.dma_start(out=bt[:], in_=b_ext[l][:, :])
                b_sb.append(bt)
            dstw_sb = cpool.tile(
                [128, gtot], F16 if (USE_F16_MSG and BATCH_ISEQ) else F32)
            nc.sync.dma_start(out=dstw_sb[:], in_=dstw_ext[:, :])

            # ---- dinv (wrapped + broadcast along free dim) ----
            degw_sb = cpool.tile([128, NW], F32)
            nc.sync.dma_start(out=degw_sb[:], in_=degw_ext[:, :])
            rcpw = cpool.tile([128, NW], F32)
            nc.vector.reciprocal(rcpw[:], degw_sb[:])
            dinv_w = cpool.tile([128, NW], F32)
            nc.scalar.activation(dinv_w[:], rcpw[:],
                                 mybir.ActivationFunctionType.Sqrt)

            # dinv_bc[p, d] = dinv[d]: per window, move the dinv column to a
            # partition-0 row (matmul vs identity), then K=1 ones-broadcast.
            dinv_bc = cpool.tile([128, NB], F32)
            for w in range(NW):
                pr = ptpool.tile([128, 128], F32, tag="tp")
                nc.tensor.matmul(pr[0:1, :], dinv_w[:, w : w + 1],
                                 ident_sb[:], start=True, stop=True)
                row_sb = wpool.tile([1, 128], F32, tag="drow")
                nc.scalar.copy(out=row_sb[:], in_=pr[0:1, :])
                pb = ptpool.tile([128, 128], F32, tag="tp")
                nc.tensor.matmul(pb[:], ones_sb[:], row_sb[:],
                                 start=True, stop=True)
                nc.scalar.copy(
                    out=dinv_bc[:, w * 128 : w * 128 + 128], in_=pb[:]
                )

            # ---- own-block ps mirror (for self-loop PSUM init) ----
            ps_own = cpool.tile([128, NB], DT_M, tag="psown")

            # ---- prologue: ps0 = dinv * x ----
            for w in range(NW):
                xt = wpool.tile([128, 128], F32, tag="xin")
                nc.sync.dma_start(out=xt[:], in_=x_ext[w * 128 : w * 128 + 128, :])
                xs = ps_own[:, w * 128 : w * 128 + 128]
                nc.vector.tensor_scalar(
                    xs, xt[:], dinv_w[:, w : w + 1], None,
                    op0=mybir.AluOpType.mult,
                )
                nc.sync.dma_start(
                    out=ps_loc[w * 128 : w * 128 + 128, :], in_=xs
                )
            allgather_ps()

            # ---- layers ----
            for layer in range(3):
                for rng_pairs in ranges:
                    # gather all chunks for this range
                    mtiles = {}
                    for k in range(nchunks):
                        g_rk = int(sum(gcell[p, k] for p in rng_pairs))
                        if g_rk == 0:
                            continue
                        g0 = int(chunk_goff[rng_pairs[0], k])
                        ni = g_rk * 128
                        it = ipool.tile([128, ni // 16], I16, tag=f"i{k}")
                        nc.sync.dma_start(
                            out=it[:],
                            in_=idx_ext[k][:, g0 * 8 : g0 * 8 + ni // 16],
                        )
                        mt = mpool.tile([128, g_rk, 128], DT_M, tag=f"m{k}")
                        nc.gpsimd.dma_gather(
                            mt[:],
                            ps_full[k][:, :],
                            it[:],
                            ni, ni, D,
                            single_packet=SINGLE_PACKET,
                            queue_num=QROT[0] % NQ,
                        )
                        QROT[0] += 1
                        mtiles[k] = (mt, g0)

                    # batched one-hot build: ONE DVE is_equal for ALL groups
                    # of this range, via stride-0 broadcast APs.
                    # s_big[:, g, d] = (iota[:, d] == dstw[:, g0r + g])
                    g0r = int(cell_goff[rng_pairs[0], 0])
                    ngr = int(sum(gcell[p, k] for p in rng_pairs
                                  for k in range(nchunks)))
                    if BATCH_ISEQ:
                        s_big = spool.tile([128, NGRMAX, PAIRW], DT_M,
                                           tag="sbig")
                        in0 = iota_sb[:].unsqueeze(1).to_broadcast(
                            [128, ngr, PAIRW])
                        in1 = dstw_sb[:, g0r : g0r + ngr].unsqueeze(
                            2).to_broadcast([128, ngr, PAIRW])
                        nc.vector.tensor_tensor(
                            s_big[:, 0:ngr, :], in0, in1,
                            op=mybir.AluOpType.is_equal,
                        )

                    for p in rng_pairs:
                        # segment-sum into PSUM [feat, PAIRW]; the self-loop
                        # term psT (unit coefficient) accumulates via PE
                        # identity matmuls after the group opens.
                        zps = pzpool.tile([128, PAIRW], F32, tag="zacc")
                        ng = int(sum(gcell[p, k] for k in range(nchunks)))
                        assert ng > 0
                        gi = 0
                        for k in range(nchunks):
                            for j in range(int(gcell[p, k])):
                                mt, g0 = mtiles[k]
                                slot = int(chunk_goff[p, k]) - g0 + j
                                gcol = int(cell_goff[p, k]) + j
                                if BATCH_ISEQ:
                                    s_t = s_big[:, gcol - g0r, :]
                                else:
                                    st = spool.tile([128, PAIRW], DT_M,
                                                    tag="s")
                                    nc.vector.tensor_scalar(
                                        st[:], iota_sb[:],
                                        dstw_sb[:, gcol : gcol + 1], None,
                                        op0=mybir.AluOpType.is_equal,
                                    )
                                    s_t = st[:]
                                # first matmul opens the accumulation group
                                # full-width (start=True); the self-term
                                # init matmuls then ACCUMULATE psT of the
                                # pair's two windows (safe regardless of
                                # whether start clears per-address or
                                # whole-bank has_written state).
                                nc.tensor.matmul(
                                    zps[:], mt[:, slot, :], s_t,
                                    start=(gi == 0),
                                    stop=(gi == ng - 1
                                          and (SELF_EDGES or ng > 1)),
                                )
                                if gi == 0 and not SELF_EDGES:
                                    for h in range(WPP):
                                        w = p * WPP + h
                                        nc.tensor.matmul(
                                            zps[:, h * 128 : h * 128 + 128],
                                            ps_own[:, w * 128 : w * 128 + 128],
                                            ident16[:],
                                            start=False,
                                            stop=(ng == 1 and h == WPP - 1),
                                        )
                                gi += 1

                        # z^T = dinv ⊙ u^T ; -> SBUF f32r (rhs of W matmul)
                        zsT = wpool.tile([128, PAIRW], DT_R, tag="zst")
                        c0 = p * PAIRW
                        nc.vector.tensor_mul(
                            zsT[:], zps[:], dinv_bc[:, c0 : c0 + PAIRW]
                        )

                        hps = phpool.tile([128, PAIRW], F32, tag="h")
                        nc.tensor.matmul(
                            hps[:], w_sb[layer][:], zsT[:],
                            start=True, stop=True,
                        )
                        hT = wpool.tile([128, PAIRW], F32, tag="ht")
                        if layer < 2:
                            nc.scalar.activation(
                                hT[:], hps[:],
                                mybir.ActivationFunctionType.Relu,
                                bias=b_sb[layer][:],
                            )
                        else:
                            nc.scalar.activation(
                                hT[:], hps[:],
                                mybir.ActivationFunctionType.Identity,
                                bias=b_sb[layer][:],
                            )
                        for h in range(WPP):
                            w = p * WPP + h
                            if w >= NW:
                                break
                            tp = ptpool.tile([128, 128], F32, tag="tp")
                            nc.tensor.transpose(
                                tp[:], hT[:, h * 128 : h * 128 + 128],
                                ident_sb[:],
                            )
                            if layer < 2:
                                pst = ps_own[:, w * 128 : w * 128 + 128]
                                nc.vector.tensor_scalar(
                                    pst, tp[:], dinv_w[:, w : w + 1], None,
                                    op0=mybir.AluOpType.mult,
                                )
                                nc.sync.dma_start(
                                    out=ps_loc[w * 128 : w * 128 + 128, :],
                                    in_=pst,
                                )
                            else:
                                ot = opool.tile([128, 128], F32, tag="oout")
                                nc.scalar.copy(out=ot[:], in_=tp[:])
                                nc.sync.dma_start(
                                    out=out_ext[w * 128 : w * 128 + 128, :],
                                    in_=ot[:],
                                )
                if layer < 2:
                    allgather_ps()

    nc.finalize()
    return nc


_CACHE = {}
TRACE = False          # set by test harness to profile + fill LAST_EXEC_NS
LAST_EXEC_NS = None


def kernel(x, edge_index, W1, b1, W2, b2, W3, b3):
    global LAST_EXEC_NS
    x = np.asarray(x, dtype=np.float32)
    edge_index = np.asarray(edge_index)
    n_nodes = x.shape[0]

    ck = (n_nodes, edge_index.shape[1],
          hash(edge_index.tobytes()))
    if ck in _CACHE:
        meta, nc = _CACHE[ck]
    else:
        meta = _preprocess(edge_index, n_nodes)
        nc = _build(meta)
        _CACHE[ck] = (meta, nc)

    NB, NW, nb_real = meta["NB"], meta["NW"], meta["nb_real"]
    nchunks = meta["nchunks"]

    iota_dt = np.float16 if USE_F16_MSG else np.float32
    iota = np.tile(np.arange(PAIRW, dtype=iota_dt), (128, 1))
    ident = np.eye(128, dtype=np.float32)
    ones1 = np.ones((1, 128), dtype=np.float32)
    ws = [np.asarray(W1, np.float32), np.asarray(W2, np.float32),
          np.asarray(W3, np.float32)]
    bs = [np.asarray(b1, np.float32), np.asarray(b2, np.float32),
          np.asarray(b3, np.float32)]

    gp_node = meta["gp_node"]
    glob_of_local = meta["glob_of_local"]
    x_pad = np.zeros((NB * NCORES, D), dtype=np.float32)
    x_pad[gp_node] = x

    in_maps = []
    for c in range(NCORES):
        xb = np.ascontiguousarray(x_pad[glob_of_local[c]])
        im = {
            "x": xb,
            "degw": meta["deg_w"][c],
            "iota": iota,
            "ident": ident,
            "ones1": ones1,
            "dstw": meta["dstw"][c].astype(
                np.float16 if (USE_F16_MSG and BATCH_ISEQ) else np.float32),
        }
        for l in range(3):
            im[f"w{l}"] = ws[l]
            im[f"b{l}"] = bs[l].reshape(128, 1)
        for k in range(nchunks):
            im[f"idx{k}"] = meta["idx_wrapped"][k][c]
        in_maps.append(im)

    res = bass_utils.run_bass_kernel_spmd(
        nc, in_maps, core_ids=list(range(NCORES)), trace=TRACE
    )
    LAST_EXEC_NS = res.exec_time_ns

    out_pad = np.empty((NB * NCORES, D), dtype=np.float32)
    for c in range(NCORES):
        out_pad[glob_of_local[c]] = res.results[c]["out"]
    return out_pad[gp_node]



# revision 65
# speedup vs baseline: 1.2054x; 1.2054x over previous
"""3-layer GCN on 8 Trainium2 NeuronCores (Bass/Tile SPMD kernel).

Strategy (sharding_hint: shard nodes + edges by destination, replicate
weights, all-gather activations between layers):

  - Nodes are split into 8 contiguous blocks (padded to a multiple of 128
    rows per core).  Core c owns destination block c.
  - Per layer, using linearity of GCN aggregation:
        out_i = [sum_{j->i} dinv_i dinv_j p_j + dinv_i^2 p_i] @ W + b
    with p = previous activations.  We store ps = dinv * p ("scaled"
    activations) so every message (including the self loop, added as an
    explicit edge) has unit coefficient:
        z_i = dinv_i * segment_sum(ps[src])       (over edges + self edges)
        out_i = z_i @ W + b ; next ps = dinv * relu(out)
  - Each core DMA-gathers ps[src] rows (512B each) for its edges from a
    replicated full-activation DRAM buffer, reduces them into per-256-dst
    "window pair" PSUM tiles with one-hot matmuls (one-hots built by DVE
    is_equal against an iota row), applies dinv, multiplies by W (PE),
    bias+ReLU (ACT), rescales, and writes its 1/8 output block.
  - An AllGather (ncfw collective) replicates the per-core ps blocks
    between layers.  Matmul operands use float32r (TF32-like, ~1e-4 rel).

Edges are sorted by (dst core, dst window pair, src chunk); each
(pair, chunk) cell is padded to whole 128-token groups with a group count
equalized across cores so a single SPMD program serves all 8 cores.
Gather indices are int16 (chunk-relative, chunks of <=32768 rows).
"""

import sys

if "/opt/trn_rl_repo" not in sys.path:
    sys.path.insert(0, "/opt/trn_rl_repo")

import numpy as np

import concourse.bacc as bacc
import concourse.mybir as mybir
import concourse.tile as tile
from concourse import bass_utils

F32 = mybir.dt.float32
F32R = mybir.dt.float32r
F16 = mybir.dt.float16
I16 = mybir.dt.int16

NCORES = 8
D = 128
USE_F32R = True      # float32r (TF32-like) matmul operands; False = exact fp32
USE_F16_MSG = True   # fp16 gathered messages + one-hot S (halves gather bytes)
import os as _os

BATCH_ISEQ = _os.environ.get("BATCH_ISEQ", "1") == "1"
SINGLE_PACKET = _os.environ.get("SINGLE_PACKET", "0") == "1"
BALANCE = _os.environ.get("BALANCE", "1") == "1"
SELF_EDGES = _os.environ.get("SELF_EDGES", "0") == "1"
NQ = 4               # SWDGE queues for gather desc-gen parallelism
PAIRW = 256          # dst window width (S matrix / PSUM free size)
WPP = PAIRW // 128   # 128-row windows per dst bin
RP = 2               # dst bins per gather range
NCHUNKS = 4          # gather source chunks = collective quarters


def _assign_balanced(dst, n_nodes, NPAIR):
    """LPT greedy: assign dst rows to (core, pair) bins equalizing total
    token count per bin (in-degree + self loop), capacity PAIRW rows.

    Per-chunk counts then vary only by the multinomial split of each
    bin's sources over the 4 chunks (sigma ~ sqrt(total)/2), vs the
    ~4x larger variance of blocked assignment."""
    import heapq

    NBINS = NCORES * NPAIR
    indeg = np.bincount(dst, minlength=n_nodes).astype(np.int64)
    if SELF_EDGES:
        indeg += 1
    order = np.argsort(-indeg, kind="stable")
    cap = np.full(NBINS, PAIRW, dtype=np.int64)
    assign = np.empty(n_nodes, dtype=np.int64)
    heap = [(0, b) for b in range(NBINS)]
    heapq.heapify(heap)
    for i in order:
        while True:
            load, b = heapq.heappop(heap)
            if cap[b] > 0:
                break
        assign[i] = b
        cap[b] -= 1
        if cap[b] > 0:
            heapq.heappush(heap, (load + int(indeg[i]), b))
    return assign


def _preprocess(edge_index, n_nodes):
    """Host-side integer preprocessing: degrees, edge partition, padding.

    Returns a dict with the static program structure (identical across
    cores) and per-core gather/one-hot metadata arrays.
    """
    src = np.asarray(edge_index[0], dtype=np.int64)
    dst = np.asarray(edge_index[1], dtype=np.int64)

    nb_real = -(-n_nodes // NCORES)              # real rows per core
    NB = -(-nb_real // 128) * 128                # padded rows per core
    NPAD = NB * NCORES
    NW = NB // 128                               # windows per core
    NPAIR = -(-NW // WPP)                        # dst bins per core
    nchunks = NCHUNKS

    deg = np.ones(n_nodes, dtype=np.float64)
    np.add.at(deg, dst, 1.0)                     # bincount, +1 self loop
    deg = deg.astype(np.float32)

    allnodes = np.arange(n_nodes, dtype=np.int64)

    if BALANCE:
        bin_of = _assign_balanced(dst, n_nodes, NPAIR)
        order_r = np.argsort(bin_of, kind="stable")
        poff_node = np.empty(n_nodes, dtype=np.int64)
        starts = np.zeros(NCORES * NPAIR + 1, dtype=np.int64)
        np.cumsum(np.bincount(bin_of, minlength=NCORES * NPAIR),
                  out=starts[1:])
        poff_node[order_r] = np.arange(n_nodes) - starts[bin_of[order_r]]
    else:
        bin_of = ((allnodes // nb_real) * NPAIR
                  + (allnodes % nb_real) // PAIRW)
        poff_node = (allnodes % nb_real) % PAIRW

    # local row -> global padded row (core-major; AllGather concat order)
    CHR = NPAD // nchunks
    csize = [CHR] * nchunks
    chunk_base = np.arange(nchunks + 1, dtype=np.int64) * CHR
    loc_r = np.arange(NB, dtype=np.int64)
    glob_of_local = np.empty((NCORES, NB), dtype=np.int64)
    for c in range(NCORES):
        glob_of_local[c] = c * NB + loc_r
    loc_node = (bin_of % NPAIR) * PAIRW + poff_node
    gp_node = glob_of_local[bin_of // NPAIR, loc_node]

    # self-loop terms are added on-chip (PSUM init transposes), not as
    # edges — unless SELF_EDGES (debug fallback to explicit self tokens)
    if SELF_EDGES:
        s_all = np.concatenate([src, allnodes])
        d_all = np.concatenate([dst, allnodes])
    else:
        s_all = src
        d_all = dst

    core = bin_of[d_all] // NPAIR
    pair = bin_of[d_all] % NPAIR
    poff = poff_node[d_all]                      # offset within bin
    sgp = gp_node[s_all]
    chunk = np.searchsorted(chunk_base, sgp, side="right") - 1
    sidx = (sgp - chunk_base[chunk]).astype(np.int64)

    # sort by (core, pair, chunk), then src within each cell (HBM locality)
    key = ((core * NPAIR) + pair) * nchunks + chunk
    order = np.lexsort((sidx, key))
    key_s = key[order]
    sidx_s = sidx[order]
    poff_s = poff[order]

    ncells = NPAIR * nchunks
    # per-core per-cell counts
    counts = np.zeros((NCORES, ncells), dtype=np.int64)
    uk, uc = np.unique(key_s, return_counts=True)
    counts.reshape(-1)[uk] = uc
    gcell = (-(-counts // 128)).max(axis=0)      # equalized group counts
    gcell = gcell.reshape(NPAIR, nchunks)        # [pair, chunk]

    gtot = int(gcell.sum())
    # stream layout: for p in pairs: for k in chunks: gcell[p,k] groups
    cell_goff = np.zeros((NPAIR, nchunks), dtype=np.int64)
    g = 0
    for p in range(NPAIR):
        for k in range(nchunks):
            cell_goff[p, k] = g
            g += gcell[p, k]

    # per-core padded token arrays in stream order (token-major flat)
    dstw_flat = np.full((NCORES, gtot * 128), -1.0, dtype=np.float32)

    # chunk stream group offsets (within each chunk's gather stream)
    chunk_goff = np.zeros((NPAIR, nchunks), dtype=np.int64)
    acc = np.zeros(nchunks, dtype=np.int64)
    for p in range(NPAIR):
        for k in range(nchunks):
            chunk_goff[p, k] = acc[k]
            acc[k] += gcell[p, k]
    gchunk = acc                                  # groups per chunk stream

    idx_streams = [
        np.zeros((NCORES, int(gchunk[k]) * 128), dtype=np.int16)
        for k in range(nchunks)
    ]

    cell_starts = np.zeros(NCORES * ncells + 1, dtype=np.int64)
    np.cumsum(counts.reshape(-1), out=cell_starts[1:])
    for c in range(NCORES):
        for p in range(NPAIR):
            for k in range(nchunks):
                cell = (c * NPAIR + p) * nchunks + k
                t0, t1 = cell_starts[cell], cell_starts[cell + 1]
                n = t1 - t0
                gk0 = chunk_goff[p, k] * 128
                idx_streams[k][c, gk0 : gk0 + n] = sidx_s[t0:t1]
                g0 = cell_goff[p, k]
                dstw_flat[c, g0 * 128 : g0 * 128 + n] = poff_s[t0:t1]
                # pads keep idx 0 / dstw -1

    # dstw: token t of group g -> [t%128, g]
    dstw = np.ascontiguousarray(
        dstw_flat.reshape(NCORES, gtot, 128).transpose(0, 2, 1)
    )

    # wrap indices: token i -> [i%16, i//16], replicated to 128 partitions
    idx_wrapped = []
    for k in range(nchunks):
        st = idx_streams[k]
        cols = st.shape[1] // 16
        w = st.reshape(NCORES, cols, 16).transpose(0, 2, 1)  # [C,16,cols]
        idx_wrapped.append(np.tile(w, (1, 8, 1)).copy())     # [C,128,cols]

    # degree layouts
    deg_pad = np.ones(NPAD, dtype=np.float32)
    deg_pad[gp_node] = deg
    deg_w = np.empty((NCORES, 128, NW), dtype=np.float32)    # wrapped
    deg_r = np.empty((NCORES, 1, NB), dtype=np.float32)      # row
    for c in range(NCORES):
        blk = deg_pad[glob_of_local[c]]
        deg_w[c] = blk.reshape(NW, 128).T
        deg_r[c, 0] = blk

    return dict(
        NB=NB, NPAD=NPAD, NW=NW, NPAIR=NPAIR, nchunks=nchunks,
        nb_real=nb_real, gcell=gcell, gtot=gtot, gchunk=gchunk,
        cell_goff=cell_goff, chunk_goff=chunk_goff,
        idx_wrapped=idx_wrapped, dstw=dstw, deg_w=deg_w, deg_r=deg_r,
        gp_node=gp_node, glob_of_local=glob_of_local,
        csize=csize,
    )


def _build(meta):
    NB, NPAD, NW, NPAIR = meta["NB"], meta["NPAD"], meta["NW"], meta["NPAIR"]
    nchunks, gcell, gtot = meta["nchunks"], meta["gcell"], meta["gtot"]
    gchunk, cell_goff, chunk_goff = (
        meta["gchunk"], meta["cell_goff"], meta["chunk_goff"],
    )

    DT_R = F32R if USE_F32R else F32
    DT_M = F16 if USE_F16_MSG else DT_R     # message/S dtype

    ranges_l = [list(range(r, min(r + RP, NPAIR)))
                for r in range(0, NPAIR, RP)]
    NGRMAX = max(int(gcell[rp, :].sum()) for rp in ranges_l)

    nc = bacc.Bacc(None, target_bir_lowering=False, num_devices=NCORES,
                   num_swdge_queues=NQ)

    x_ext = nc.dram_tensor("x", [NB, D], F32, kind="ExternalInput")
    degw_ext = nc.dram_tensor("degw", [128, NW], F32, kind="ExternalInput")
    iota_ext = nc.dram_tensor("iota", [128, PAIRW],
                              F16 if USE_F16_MSG else F32,
                              kind="ExternalInput")
    ident_ext = nc.dram_tensor("ident", [128, 128], F32, kind="ExternalInput")
    ones_ext = nc.dram_tensor("ones1", [1, 128], F32, kind="ExternalInput")
    w_ext = [
        nc.dram_tensor(f"w{l}", [D, D], F32, kind="ExternalInput")
        for l in range(3)
    ]
    b_ext = [
        nc.dram_tensor(f"b{l}", [128, 1], F32, kind="ExternalInput")
        for l in range(3)
    ]
    idx_ext = [
        nc.dram_tensor(f"idx{k}", [128, int(gchunk[k]) * 8], I16,
                       kind="ExternalInput")
        for k in range(nchunks)
    ]
    dstw_ext = nc.dram_tensor("dstw", [128, gtot],
                              F16 if (USE_F16_MSG and BATCH_ISEQ) else F32,
                              kind="ExternalInput")
    out_ext = nc.dram_tensor("out", [NB, D], F32, kind="ExternalOutput")

    csize = meta["csize"]
    cb = np.concatenate([[0], np.cumsum(csize)]).astype(np.int64)
    ps_loc = nc.dram_tensor("ps_loc", [NB, D], DT_M)
    ps_full_all = nc.dram_tensor("ps_full", [NPAD, D], DT_M,
                                 addr_space="Shared")
    ps_full = [ps_full_all[int(cb[k]) : int(cb[k + 1]), :]
               for k in range(nchunks)]

    def allgather_ps():
        nc.gpsimd.collective_compute(
            "AllGather", mybir.AluOpType.bypass,
            replica_groups=[list(range(NCORES))],
            ins=[ps_loc.ap().opt()], outs=[ps_full_all.ap().opt()],
        )

    QROT = [0]

    # gather ranges: RP pairs each
    ranges = [list(range(r, min(r + RP, NPAIR))) for r in range(0, NPAIR, RP)]

    with tile.TileContext(nc) as tc:
        with (
            tc.tile_pool(name="const", bufs=1) as cpool,
            tc.tile_pool(name="msg", bufs=2) as mpool,
            tc.tile_pool(name="idxp", bufs=2) as ipool,
            tc.tile_pool(name="sbld", bufs=2) as spool,
            tc.tile_pool(name="work", bufs=3) as wpool,
            tc.tile_pool(name="outp", bufs=4) as opool,
            tc.tile_pool(name="pz", bufs=2, space="PSUM") as pzpool,
            tc.tile_pool(name="pt", bufs=2, space="PSUM") as ptpool,
            tc.tile_pool(name="ph", bufs=2, space="PSUM") as phpool,
        ):
            # ---- constants ----
            iota_sb = cpool.tile([128, PAIRW], F16 if USE_F16_MSG else F32)
            nc.sync.dma_start(out=iota_sb[:], in_=iota_ext[:, :])
            ident_sb = cpool.tile([128, 128], F32)
            nc.sync.dma_start(out=ident_sb[:], in_=ident_ext[:, :])
            ident16 = cpool.tile([128, 128], DT_M, tag="ident16")
            nc.vector.tensor_copy(ident16[:], ident_sb[:])
            ones_sb = cpool.tile([1, 128], F32)
            nc.sync.dma_start(out=ones_sb[:], in_=ones_ext[:, :])
            w_sb = []
            for l in range(3):
                wt = cpool.tile([D, D], F32, tag=f"wraw{l}")
                nc.sync.dma_start(out=wt[:], in_=w_ext[l][:, :])
                if USE_F32R:
                    wr = cpool.tile([D, D], F32R, tag=f"w{l}")
                    nc.vector.tensor_copy(wr[:], wt[:])
                    w_sb.append(wr)
                else:
                    w_sb.append(wt)
            b_sb = []
            for l in range(3):
                bt = cpool.tile([128, 1], F32, tag=f"b{l}")
                nc.sync.dma_start(out=bt[:], in_=b_ext[l][:, :])
                b_sb.append(bt)
            dstw_sb = cpool.tile(
                [128, gtot], F16 if (USE_F16_MSG and BATCH_ISEQ) else F32)
            nc.sync.dma_start(out=dstw_sb[:], in_=dstw_ext[:, :])

            # ---- dinv (wrapped + broadcast along free dim) ----
            degw_sb = cpool.tile([128, NW], F32)
            nc.sync.dma_start(out=degw_sb[:], in_=degw_ext[:, :])
            rcpw = cpool.tile([128, NW], F32)
            nc.vector.reciprocal(rcpw[:], degw_sb[:])
            dinv_w = cpool.tile([128, NW], F32)
            nc.scalar.activation(dinv_w[:], rcpw[:],
                                 mybir.ActivationFunctionType.Sqrt)

            # dinv_bc[p, d] = dinv[d]: per window, move the dinv column to a
            # partition-0 row (matmul vs identity), then K=1 ones-broadcast.
            dinv_bc = cpool.tile([128, NB], F32)
            for w in range(NW):
                pr = ptpool.tile([128, 128], F32, tag="tp")
                nc.tensor.matmul(pr[0:1, :], dinv_w[:, w : w + 1],
                                 ident_sb[:], start=True, stop=True)
                row_sb = wpool.tile([1, 128], F32, tag="drow")
                nc.scalar.copy(out=row_sb[:], in_=pr[0:1, :])
                pb = ptpool.tile([128, 128], F32, tag="tp")
                nc.tensor.matmul(pb[:], ones_sb[:], row_sb[:],
                                 start=True, stop=True)
                nc.scalar.copy(
                    out=dinv_bc[:, w * 128 : w * 128 + 128], in_=pb[:]
                )

            # ---- own-block ps mirror (for self-loop PSUM init) ----
            ps_own = cpool.tile([128, NB], DT_M, tag="psown")

            # ---- prologue: ps0 = dinv * x ----
            for w in range(NW):
                xt = wpool.tile([128, 128], F32, tag="xin")
                nc.sync.dma_start(out=xt[:], in_=x_ext[w * 128 : w * 128 + 128, :])
                xs = ps_own[:, w * 128 : w * 128 + 128]
                nc.vector.tensor_scalar(
                    xs, xt[:], dinv_w[:, w : w + 1], None,
                    op0=mybir.AluOpType.mult,
                )
                nc.sync.dma_start(
                    out=ps_loc[w * 128 : w * 128 + 128, :], in_=xs
                )
            allgather_ps()

            # ---- layers ----
            for layer in range(3):
                for rng_pairs in ranges:
                    # gather all chunks for this range
                    mtiles = {}
                    for k in range(nchunks):
                        g_rk = int(sum(gcell[p, k] for p in rng_pairs))
                        if g_rk == 0:
                            continue
                        g0 = int(chunk_goff[rng_pairs[0], k])
                        ni = g_rk * 128
                        it = ipool.tile([128, ni // 16], I16, tag=f"i{k}")
                        nc.sync.dma_start(
                            out=it[:],
                            in_=idx_ext[k][:, g0 * 8 : g0 * 8 + ni // 16],
                        )
                        mt = mpool.tile([128, g_rk, 128], DT_M, tag=f"m{k}")
                        nc.gpsimd.dma_gather(
                            mt[:],
                            ps_full[k][:, :],
                            it[:],
                            ni, ni, D,
                            single_packet=SINGLE_PACKET,
                            queue_num=QROT[0] % NQ,
                        )
                        QROT[0] += 1
                        mtiles[k] = (mt, g0)

                    # batched one-hot build: ONE DVE is_equal for ALL groups
                    # of this range, via stride-0 broadcast APs.
                    # s_big[:, g, d] = (iota[:, d] == dstw[:, g0r + g])
                    g0r = int(cell_goff[rng_pairs[0], 0])
                    ngr = int(sum(gcell[p, k] for p in rng_pairs
                                  for k in range(nchunks)))
                    if BATCH_ISEQ:
                        s_big = spool.tile([128, NGRMAX, PAIRW], DT_M,
                                           tag="sbig")
                        in0 = iota_sb[:].unsqueeze(1).to_broadcast(
                            [128, ngr, PAIRW])
                        in1 = dstw_sb[:, g0r : g0r + ngr].unsqueeze(
                            2).to_broadcast([128, ngr, PAIRW])
                        nc.vector.tensor_tensor(
                            s_big[:, 0:ngr, :], in0, in1,
                            op=mybir.AluOpType.is_equal,
                        )

                    for p in rng_pairs:
                        # segment-sum into PSUM [feat, PAIRW]; the self-loop
                        # term psT (unit coefficient) accumulates via PE
                        # identity matmuls after the group opens.
                        zps = pzpool.tile([128, PAIRW], F32, tag="zacc")
                        ng = int(sum(gcell[p, k] for k in range(nchunks)))
                        assert ng > 0
                        gi = 0
                        for k in range(nchunks):
                            for j in range(int(gcell[p, k])):
                                mt, g0 = mtiles[k]
                                slot = int(chunk_goff[p, k]) - g0 + j
                                gcol = int(cell_goff[p, k]) + j
                                if BATCH_ISEQ:
                                    s_t = s_big[:, gcol - g0r, :]
                                else:
                                    st = spool.tile([128, PAIRW], DT_M,
                                                    tag="s")
                                    nc.vector.tensor_scalar(
                                        st[:], iota_sb[:],
                                        dstw_sb[:, gcol : gcol + 1], None,
                                        op0=mybir.AluOpType.is_equal,
                                    )
                                    s_t = st[:]
                                # first matmul opens the accumulation group
                                # full-width (start=True); the self-term
                                # init matmuls then ACCUMULATE psT of the
                                # pair's two windows (safe regardless of
                                # whether start clears per-address or
                                # whole-bank has_written state).
                                nc.tensor.matmul(
                                    zps[:], mt[:, slot, :], s_t,
                                    start=(gi == 0),
                                    stop=(gi == ng - 1
                                          and (SELF_EDGES or ng > 1)),
                                )
                                if gi == 0 and not SELF_EDGES:
                                    for h in range(WPP):
                                        w = p * WPP + h
                                        nc.tensor.matmul(
                                            zps[:, h * 128 : h * 128 + 128],
                                            ps_own[:, w * 128 : w * 128 + 128],
                                            ident16[:],
                                            start=False,
                                            stop=(ng == 1 and h == WPP - 1),
                                        )
                                gi += 1

                        # z^T = dinv ⊙ u^T ; -> SBUF f32r (rhs of W matmul)
                        zsT = wpool.tile([128, PAIRW], DT_R, tag="zst")
                        c0 = p * PAIRW
                        nc.vector.tensor_mul(
                            zsT[:], zps[:], dinv_bc[:, c0 : c0 + PAIRW]
                        )

                        hps = phpool.tile([128, PAIRW], F32, tag="h")
                        nc.tensor.matmul(
                            hps[:], w_sb[layer][:], zsT[:],
                            start=True, stop=True,
                        )
                        hT = wpool.tile([128, PAIRW], F32, tag="ht")
                        if layer < 2:
                            nc.scalar.activation(
                                hT[:], hps[:],
                                mybir.ActivationFunctionType.Relu,
                                bias=b_sb[layer][:],
                            )
                        else:
                            nc.scalar.activation(
                                hT[:], hps[:],
                                mybir.ActivationFunctionType.Identity,
                                bias=b_sb[layer][:],
                            )
                        for h in range(WPP):
                            w = p * WPP + h
                            if w >= NW:
                                break
                            tp = ptpool.tile([128, 128], F32, tag="tp")
                            nc.tensor.transpose(
                                tp[:], hT[:, h * 128 : h * 128 + 128],
                                ident_sb[:],
                            )
                            if layer < 2:
                                pst = ps_own[:, w * 128 : w * 128 + 128]
                                nc.vector.tensor_scalar(
                                    pst, tp[:], dinv_w[:, w : w + 1], None,
                                    op0=mybir.AluOpType.mult,
                                )
                                nc.sync.dma_start(
                                    out=ps_loc[w * 128 : w * 128 + 128, :],
                                    in_=pst,
                                )
                            else:
                                ot = opool.tile([128, 128], F32, tag="oout")
                                nc.scalar.copy(out=ot[:], in_=tp[:])
                                nc.sync.dma_start(
                                    out=out_ext[w * 128 : w * 128 + 128, :],
                                    in_=ot[:],
                                )
                if layer < 2:
                    allgather_ps()

    nc.finalize()
    return nc


_CACHE = {}
TRACE = False          # set by test harness to profile + fill LAST_EXEC_NS
LAST_EXEC_NS = None


def kernel(x, edge_index, W1, b1, W2, b2, W3, b3):
    global LAST_EXEC_NS
    x = np.asarray(x, dtype=np.float32)
    edge_index = np.asarray(edge_index)
    n_nodes = x.shape[0]

    ck = (n_nodes, edge_index.shape[1],
          hash(edge_index.tobytes()))
    if ck in _CACHE:
        meta, nc = _CACHE[ck]
    else:
        meta = _preprocess(edge_index, n_nodes)
        nc = _build(meta)
        _CACHE[ck] = (meta, nc)

    NB, NW, nb_real = meta["NB"], meta["NW"], meta["nb_real"]
    nchunks = meta["nchunks"]

    iota_dt = np.float16 if USE_F16_MSG else np.float32
    iota = np.tile(np.arange(PAIRW, dtype=iota_dt), (128, 1))
    ident = np.eye(128, dtype=np.float32)
    ones1 = np.ones((1, 128), dtype=np.float32)
    ws = [np.asarray(W1, np.float32), np.asarray(W2, np.float32),
          np.asarray(W3, np.float32)]
    bs = [np.asarray(b1, np.float32), np.asarray(b2, np.float32),
          np.asarray(b3, np.float32)]

    gp_node = meta["gp_node"]
    glob_of_local = meta["glob_of_local"]
    x_pad = np.zeros((NB * NCORES, D), dtype=np.float32)
    x_pad[gp_node] = x

    in_maps = []
    for c in range(NCORES):
        xb = np.ascontiguousarray(x_pad[glob_of_local[c]])
        im = {
            "x": xb,
            "degw": meta["deg_w"][c],
            "iota": iota,
            "ident": ident,
            "ones1": ones1,
            "dstw": meta["dstw"][c].astype(
                np.float16 if (USE_F16_MSG and BATCH_ISEQ) else np.float32),
        }
        for l in range(3):
            im[f"w{l}"] = ws[l]
            im[f"b{l}"] = bs[l].reshape(128, 1)
        for k in range(nchunks):
            im[f"idx{k}"] = meta["idx_wrapped"][k][c]
        in_maps.append(im)

    res = bass_utils.run_bass_kernel_spmd(
        nc, in_maps, core_ids=list(range(NCORES)), trace=TRACE
    )
    LAST_EXEC_NS = res.exec_time_ns

    out_pad = np.empty((NB * NCORES, D), dtype=np.float32)
    for c in range(NCORES):
        out_pad[glob_of_local[c]] = res.results[c]["out"]
    return out_pad[gp_node]



# revision 67
# speedup vs baseline: 1.2406x; 1.0292x over previous
"""3-layer GCN on 8 Trainium2 NeuronCores (Bass/Tile SPMD kernel).

Strategy (sharding_hint: shard nodes + edges by destination, replicate
weights, all-gather activations between layers):

  - Nodes are split into 8 contiguous blocks (padded to a multiple of 128
    rows per core).  Core c owns destination block c.
  - Per layer, using linearity of GCN aggregation:
        out_i = [sum_{j->i} dinv_i dinv_j p_j + dinv_i^2 p_i] @ W + b
    with p = previous activations.  We store ps = dinv * p ("scaled"
    activations) so every message (including the self loop, added as an
    explicit edge) has unit coefficient:
        z_i = dinv_i * segment_sum(ps[src])       (over edges + self edges)
        out_i = z_i @ W + b ; next ps = dinv * relu(out)
  - Each core DMA-gathers ps[src] rows (512B each) for its edges from a
    replicated full-activation DRAM buffer, reduces them into per-256-dst
    "window pair" PSUM tiles with one-hot matmuls (one-hots built by DVE
    is_equal against an iota row), applies dinv, multiplies by W (PE),
    bias+ReLU (ACT), rescales, and writes its 1/8 output block.
  - An AllGather (ncfw collective) replicates the per-core ps blocks
    between layers.  Matmul operands use float32r (TF32-like, ~1e-4 rel).

Edges are sorted by (dst core, dst window pair, src chunk); each
(pair, chunk) cell is padded to whole 128-token groups with a group count
equalized across cores so a single SPMD program serves all 8 cores.
Gather indices are int16 (chunk-relative, chunks of <=32768 rows).
"""

import sys

if "/opt/trn_rl_repo" not in sys.path:
    sys.path.insert(0, "/opt/trn_rl_repo")

import numpy as np

import concourse.bacc as bacc
import concourse.mybir as mybir
import concourse.tile as tile
from concourse import bass_utils

F32 = mybir.dt.float32
F32R = mybir.dt.float32r
F16 = mybir.dt.float16
I16 = mybir.dt.int16

NCORES = 8
D = 128
USE_F32R = True      # float32r (TF32-like) matmul operands; False = exact fp32
USE_F16_MSG = True   # fp16 gathered messages + one-hot S (halves gather bytes)
import os as _os

BATCH_ISEQ = _os.environ.get("BATCH_ISEQ", "1") == "1"
SINGLE_PACKET = _os.environ.get("SINGLE_PACKET", "0") == "1"
BALANCE = _os.environ.get("BALANCE", "1") == "1"
SELF_EDGES = _os.environ.get("SELF_EDGES", "0") == "1"
NQ = 4               # SWDGE queues for gather desc-gen parallelism
PAIRW = 256          # dst window width (S matrix / PSUM free size)
WPP = PAIRW // 128   # 128-row windows per dst bin
RP = 2               # dst bins per gather range
NCHUNKS = 4          # gather source chunks = collective quarters


def _assign_balanced(dst, n_nodes, NPAIR):
    """LPT greedy: assign dst rows to (core, pair) bins equalizing total
    token count per bin (in-degree + self loop), capacity PAIRW rows.

    Per-chunk counts then vary only by the multinomial split of each
    bin's sources over the 4 chunks (sigma ~ sqrt(total)/2), vs the
    ~4x larger variance of blocked assignment."""
    import heapq

    NBINS = NCORES * NPAIR
    indeg = np.bincount(dst, minlength=n_nodes).astype(np.int64)
    if SELF_EDGES:
        indeg += 1
    order = np.argsort(-indeg, kind="stable")
    cap = np.full(NBINS, PAIRW, dtype=np.int64)
    assign = np.empty(n_nodes, dtype=np.int64)
    heap = [(0, b) for b in range(NBINS)]
    heapq.heapify(heap)
    for i in order:
        while True:
            load, b = heapq.heappop(heap)
            if cap[b] > 0:
                break
        assign[i] = b
        cap[b] -= 1
        if cap[b] > 0:
            heapq.heappush(heap, (load + int(indeg[i]), b))
    return assign


def _preprocess(edge_index, n_nodes):
    """Host-side integer preprocessing: degrees, edge partition, padding.

    Returns a dict with the static program structure (identical across
    cores) and per-core gather/one-hot metadata arrays.
    """
    src = np.asarray(edge_index[0], dtype=np.int64)
    dst = np.asarray(edge_index[1], dtype=np.int64)

    nb_real = -(-n_nodes // NCORES)              # real rows per core
    NB = -(-nb_real // 128) * 128                # padded rows per core
    NPAD = NB * NCORES
    NW = NB // 128                               # windows per core
    NPAIR = -(-NW // WPP)                        # dst bins per core
    nchunks = NCHUNKS

    deg = np.ones(n_nodes, dtype=np.float64)
    np.add.at(deg, dst, 1.0)                     # bincount, +1 self loop
    deg = deg.astype(np.float32)

    allnodes = np.arange(n_nodes, dtype=np.int64)

    if BALANCE:
        bin_of = _assign_balanced(dst, n_nodes, NPAIR)
        order_r = np.argsort(bin_of, kind="stable")
        poff_node = np.empty(n_nodes, dtype=np.int64)
        starts = np.zeros(NCORES * NPAIR + 1, dtype=np.int64)
        np.cumsum(np.bincount(bin_of, minlength=NCORES * NPAIR),
                  out=starts[1:])
        poff_node[order_r] = np.arange(n_nodes) - starts[bin_of[order_r]]
    else:
        bin_of = ((allnodes // nb_real) * NPAIR
                  + (allnodes % nb_real) // PAIRW)
        poff_node = (allnodes % nb_real) % PAIRW

    # local row -> global padded row (core-major; AllGather concat order)
    CHR = NPAD // nchunks
    csize = [CHR] * nchunks
    chunk_base = np.arange(nchunks + 1, dtype=np.int64) * CHR
    loc_r = np.arange(NB, dtype=np.int64)
    glob_of_local = np.empty((NCORES, NB), dtype=np.int64)
    for c in range(NCORES):
        glob_of_local[c] = c * NB + loc_r
    loc_node = (bin_of % NPAIR) * PAIRW + poff_node
    gp_node = glob_of_local[bin_of // NPAIR, loc_node]

    # self-loop terms are added on-chip (PSUM init transposes), not as
    # edges — unless SELF_EDGES (debug fallback to explicit self tokens)
    if SELF_EDGES:
        s_all = np.concatenate([src, allnodes])
        d_all = np.concatenate([dst, allnodes])
    else:
        s_all = src
        d_all = dst

    core = bin_of[d_all] // NPAIR
    pair = bin_of[d_all] % NPAIR
    poff = poff_node[d_all]                      # offset within bin
    sgp = gp_node[s_all]
    chunk = np.searchsorted(chunk_base, sgp, side="right") - 1
    sidx = (sgp - chunk_base[chunk]).astype(np.int64)

    # sort by (core, pair, chunk), then src within each cell (HBM locality)
    key = ((core * NPAIR) + pair) * nchunks + chunk
    order = np.lexsort((sidx, key))
    key_s = key[order]
    sidx_s = sidx[order]
    poff_s = poff[order]

    ncells = NPAIR * nchunks
    # per-core per-cell counts
    counts = np.zeros((NCORES, ncells), dtype=np.int64)
    uk, uc = np.unique(key_s, return_counts=True)
    counts.reshape(-1)[uk] = uc
    gcell = (-(-counts // 128)).max(axis=0)      # equalized group counts
    gcell = gcell.reshape(NPAIR, nchunks)        # [pair, chunk]

    gtot = int(gcell.sum())
    # stream layout: for p in pairs: for k in chunks: gcell[p,k] groups
    cell_goff = np.zeros((NPAIR, nchunks), dtype=np.int64)
    g = 0
    for p in range(NPAIR):
        for k in range(nchunks):
            cell_goff[p, k] = g
            g += gcell[p, k]

    # per-core padded token arrays in stream order (token-major flat)
    dstw_flat = np.full((NCORES, gtot * 128), -1.0, dtype=np.float32)

    # chunk stream group offsets (within each chunk's gather stream)
    chunk_goff = np.zeros((NPAIR, nchunks), dtype=np.int64)
    acc = np.zeros(nchunks, dtype=np.int64)
    for p in range(NPAIR):
        for k in range(nchunks):
            chunk_goff[p, k] = acc[k]
            acc[k] += gcell[p, k]
    gchunk = acc                                  # groups per chunk stream

    idx_streams = [
        np.zeros((NCORES, int(gchunk[k]) * 128), dtype=np.int16)
        for k in range(nchunks)
    ]

    cell_starts = np.zeros(NCORES * ncells + 1, dtype=np.int64)
    np.cumsum(counts.reshape(-1), out=cell_starts[1:])
    for c in range(NCORES):
        for p in range(NPAIR):
            for k in range(nchunks):
                cell = (c * NPAIR + p) * nchunks + k
                t0, t1 = cell_starts[cell], cell_starts[cell + 1]
                n = t1 - t0
                gk0 = chunk_goff[p, k] * 128
                idx_streams[k][c, gk0 : gk0 + n] = sidx_s[t0:t1]
                g0 = cell_goff[p, k]
                dstw_flat[c, g0 * 128 : g0 * 128 + n] = poff_s[t0:t1]
                # pads keep idx 0 / dstw -1

    # dstw: token t of group g -> [t%128, g]
    dstw = np.ascontiguousarray(
        dstw_flat.reshape(NCORES, gtot, 128).transpose(0, 2, 1)
    )

    # wrap indices: token i -> [i%16, i//16], replicated to 128 partitions
    idx_wrapped = []
    for k in range(nchunks):
        st = idx_streams[k]
        cols = st.shape[1] // 16
        w = st.reshape(NCORES, cols, 16).transpose(0, 2, 1)  # [C,16,cols]
        idx_wrapped.append(np.tile(w, (1, 8, 1)).copy())     # [C,128,cols]

    # degree layouts
    deg_pad = np.ones(NPAD, dtype=np.float32)
    deg_pad[gp_node] = deg
    deg_w = np.empty((NCORES, 128, NW), dtype=np.float32)    # wrapped
    deg_r = np.empty((NCORES, 1, NB), dtype=np.float32)      # row
    for c in range(NCORES):
        blk = deg_pad[glob_of_local[c]]
        deg_w[c] = blk.reshape(NW, 128).T
        deg_r[c, 0] = blk

    return dict(
        NB=NB, NPAD=NPAD, NW=NW, NPAIR=NPAIR, nchunks=nchunks,
        nb_real=nb_real, gcell=gcell, gtot=gtot, gchunk=gchunk,
        cell_goff=cell_goff, chunk_goff=chunk_goff,
        idx_wrapped=idx_wrapped, dstw=dstw, deg_w=deg_w, deg_r=deg_r,
        gp_node=gp_node, glob_of_local=glob_of_local,
        csize=csize,
    )


def _build(meta):
    NB, NPAD, NW, NPAIR = meta["NB"], meta["NPAD"], meta["NW"], meta["NPAIR"]
    nchunks, gcell, gtot = meta["nchunks"], meta["gcell"], meta["gtot"]
    gchunk, cell_goff, chunk_goff = (
        meta["gchunk"], meta["cell_goff"], meta["chunk_goff"],
    )

    DT_R = F32R if USE_F32R else F32
    DT_M = F16 if USE_F16_MSG else DT_R     # message/S dtype

    ranges_l = [list(range(r, min(r + RP, NPAIR)))
                for r in range(0, NPAIR, RP)]
    NGRMAX = max(int(gcell[rp, :].sum()) for rp in ranges_l)

    nc = bacc.Bacc(None, target_bir_lowering=False, num_devices=NCORES,
                   num_swdge_queues=NQ)

    x_ext = nc.dram_tensor("x", [NB, D], F32, kind="ExternalInput")
    degw_ext = nc.dram_tensor("degw", [128, NW], F32, kind="ExternalInput")
    iota_ext = nc.dram_tensor("iota", [128, PAIRW],
                              F16 if USE_F16_MSG else F32,
                              kind="ExternalInput")
    ident_ext = nc.dram_tensor("ident", [128, 128], F32, kind="ExternalInput")
    ones_ext = nc.dram_tensor("ones1", [1, 128], F32, kind="ExternalInput")
    w_ext = [
        nc.dram_tensor(f"w{l}", [D, D], F32, kind="ExternalInput")
        for l in range(3)
    ]
    b_ext = [
        nc.dram_tensor(f"b{l}", [128, 1], F32, kind="ExternalInput")
        for l in range(3)
    ]
    idx_ext = [
        nc.dram_tensor(f"idx{k}", [128, int(gchunk[k]) * 8], I16,
                       kind="ExternalInput")
        for k in range(nchunks)
    ]
    dstw_ext = nc.dram_tensor("dstw", [128, gtot],
                              F16 if (USE_F16_MSG and BATCH_ISEQ) else F32,
                              kind="ExternalInput")
    out_ext = nc.dram_tensor("out", [NB, D], F32, kind="ExternalOutput")

    csize = meta["csize"]
    cb = np.concatenate([[0], np.cumsum(csize)]).astype(np.int64)
    ps_loc = nc.dram_tensor("ps_loc", [NB, D], DT_M)
    ps_full_all = nc.dram_tensor("ps_full", [NPAD, D], DT_M,
                                 addr_space="Shared")
    ps_full = [ps_full_all[int(cb[k]) : int(cb[k + 1]), :]
               for k in range(nchunks)]

    def allgather_ps():
        nc.gpsimd.collective_compute(
            "AllGather", mybir.AluOpType.bypass,
            replica_groups=[list(range(NCORES))],
            ins=[ps_loc.ap().opt()], outs=[ps_full_all.ap().opt()],
        )

    QROT = [0]

    # gather ranges: RP pairs each
    ranges = [list(range(r, min(r + RP, NPAIR))) for r in range(0, NPAIR, RP)]

    with tile.TileContext(nc) as tc:
        with (
            tc.tile_pool(name="const", bufs=1) as cpool,
            tc.tile_pool(name="msg", bufs=2) as mpool,
            tc.tile_pool(name="idxp", bufs=2) as ipool,
            tc.tile_pool(name="sbld", bufs=2) as spool,
            tc.tile_pool(name="work", bufs=3) as wpool,
            tc.tile_pool(name="outp", bufs=4) as opool,
            tc.tile_pool(name="pz", bufs=2, space="PSUM") as pzpool,
            tc.tile_pool(name="pt", bufs=2, space="PSUM") as ptpool,
            tc.tile_pool(name="ph", bufs=2, space="PSUM") as phpool,
        ):
            # ---- constants ----
            iota_sb = cpool.tile([128, PAIRW], F16 if USE_F16_MSG else F32)
            nc.sync.dma_start(out=iota_sb[:], in_=iota_ext[:, :])
            ident_sb = cpool.tile([128, 128], F32)
            nc.sync.dma_start(out=ident_sb[:], in_=ident_ext[:, :])
            ident16 = cpool.tile([128, 128], DT_M, tag="ident16")
            nc.vector.tensor_copy(ident16[:], ident_sb[:])
            ones_sb = cpool.tile([1, 128], F32)
            nc.sync.dma_start(out=ones_sb[:], in_=ones_ext[:, :])
            w_sb = []
            for l in range(3):
                wt = cpool.tile([D, D], F32, tag=f"wraw{l}")
                nc.sync.dma_start(out=wt[:], in_=w_ext[l][:, :])
                if USE_F32R:
                    wr = cpool.tile([D, D], F32R, tag=f"w{l}")
                    nc.vector.tensor_copy(wr[:], wt[:])
                    w_sb.append(wr)
                else:
                    w_sb.append(wt)
            b_sb = []
            for l in range(3):
                bt = cpool.tile([128, 1], F32, tag=f"b{l}")
                nc.sync.dma_start(out=bt[:], in_=b_ext[l][:, :])
                b_sb.append(bt)
            dstw_sb = cpool.tile(
                [128, gtot], F16 if (USE_F16_MSG and BATCH_ISEQ) else F32)
            nc.sync.dma_start(out=dstw_sb[:], in_=dstw_ext[:, :])

            # ---- dinv (wrapped + broadcast along free dim) ----
            degw_sb = cpool.tile([128, NW], F32)
            nc.sync.dma_start(out=degw_sb[:], in_=degw_ext[:, :])
            rcpw = cpool.tile([128, NW], F32)
            nc.vector.reciprocal(rcpw[:], degw_sb[:])
            dinv_w = cpool.tile([128, NW], F32)
            nc.scalar.activation(dinv_w[:], rcpw[:],
                                 mybir.ActivationFunctionType.Sqrt)

            # dinv_bc[p, d] = dinv[d]: per window, move the dinv column to a
            # partition-0 row (matmul vs identity), then K=1 ones-broadcast.
            dinv_bc = cpool.tile([128, NB], DT_M)
            for w in range(NW):
                pr = ptpool.tile([128, 128], F32, tag="tp")
                nc.tensor.matmul(pr[0:1, :], dinv_w[:, w : w + 1],
                                 ident_sb[:], start=True, stop=True)
                row_sb = wpool.tile([1, 128], F32, tag="drow")
                nc.scalar.copy(out=row_sb[:], in_=pr[0:1, :])
                pb = ptpool.tile([128, 128], F32, tag="tp")
                nc.tensor.matmul(pb[:], ones_sb[:], row_sb[:],
                                 start=True, stop=True)
                nc.scalar.copy(
                    out=dinv_bc[:, w * 128 : w * 128 + 128], in_=pb[:]
                )

            # ---- own-block ps mirror (for self-loop PSUM init) ----
            # double-buffered across layers: layer L reads ps in buffer
            # L %% 2 while its epilogue writes the next ps into (L+1) %% 2 —
            # no write-after-read window on a live buffer.
            ps_own_a = cpool.tile([128, NB], DT_M, tag="psown0")
            ps_own_b = cpool.tile([128, NB], DT_M, tag="psown1")
            ps_own2 = [ps_own_a, ps_own_b]
            ps_own = ps_own_a

            # ---- prologue: ps0 = dinv * x ----
            for w in range(NW):
                xt = wpool.tile([128, 128], F32, tag="xin")
                nc.sync.dma_start(out=xt[:], in_=x_ext[w * 128 : w * 128 + 128, :])
                xs = ps_own[:, w * 128 : w * 128 + 128]
                nc.vector.tensor_scalar(
                    xs, xt[:], dinv_w[:, w : w + 1], None,
                    op0=mybir.AluOpType.mult,
                )
                nc.sync.dma_start(
                    out=ps_loc[w * 128 : w * 128 + 128, :], in_=xs
                )
            allgather_ps()

            # ---- layers ----
            for layer in range(3):
                for rng_pairs in ranges:
                    # gather all chunks for this range
                    mtiles = {}
                    for k in range(nchunks):
                        g_rk = int(sum(gcell[p, k] for p in rng_pairs))
                        if g_rk == 0:
                            continue
                        g0 = int(chunk_goff[rng_pairs[0], k])
                        ni = g_rk * 128
                        it = ipool.tile([128, ni // 16], I16, tag=f"i{k}")
                        nc.sync.dma_start(
                            out=it[:],
                            in_=idx_ext[k][:, g0 * 8 : g0 * 8 + ni // 16],
                        )
                        mt = mpool.tile([128, g_rk, 128], DT_M, tag=f"m{k}")
                        nc.gpsimd.dma_gather(
                            mt[:],
                            ps_full[k][:, :],
                            it[:],
                            ni, ni, D,
                            single_packet=SINGLE_PACKET,
                            queue_num=QROT[0] % NQ,
                        )
                        QROT[0] += 1
                        mtiles[k] = (mt, g0)

                    # batched one-hot build: ONE DVE is_equal for ALL groups
                    # of this range, via stride-0 broadcast APs.
                    # s_big[:, g, d] = (iota[:, d] == dstw[:, g0r + g])
                    g0r = int(cell_goff[rng_pairs[0], 0])
                    ngr = int(sum(gcell[p, k] for p in rng_pairs
                                  for k in range(nchunks)))
                    if BATCH_ISEQ:
                        s_big = spool.tile([128, NGRMAX, PAIRW], DT_M,
                                           tag="sbig")
                        in0 = iota_sb[:].unsqueeze(1).to_broadcast(
                            [128, ngr, PAIRW])
                        in1 = dstw_sb[:, g0r : g0r + ngr].unsqueeze(
                            2).to_broadcast([128, ngr, PAIRW])
                        nc.vector.tensor_tensor(
                            s_big[:, 0:ngr, :], in0, in1,
                            op=mybir.AluOpType.is_equal,
                        )

                    for p in rng_pairs:
                        # segment-sum into PSUM [feat, PAIRW]; the self-loop
                        # term psT (unit coefficient) accumulates via PE
                        # identity matmuls after the group opens.
                        zps = pzpool.tile([128, PAIRW], F32, tag="zacc")
                        ng = int(sum(gcell[p, k] for k in range(nchunks)))
                        assert ng > 0
                        gi = 0
                        for k in range(nchunks):
                            for j in range(int(gcell[p, k])):
                                mt, g0 = mtiles[k]
                                slot = int(chunk_goff[p, k]) - g0 + j
                                gcol = int(cell_goff[p, k]) + j
                                if BATCH_ISEQ:
                                    s_t = s_big[:, gcol - g0r, :]
                                else:
                                    st = spool.tile([128, PAIRW], DT_M,
                                                    tag="s")
                                    nc.vector.tensor_scalar(
                                        st[:], iota_sb[:],
                                        dstw_sb[:, gcol : gcol + 1], None,
                                        op0=mybir.AluOpType.is_equal,
                                    )
                                    s_t = st[:]
                                # first matmul opens the accumulation group
                                # full-width (start=True); the self-term
                                # init matmuls then ACCUMULATE psT of the
                                # pair's two windows (safe regardless of
                                # whether start clears per-address or
                                # whole-bank has_written state).
                                nc.tensor.matmul(
                                    zps[:], mt[:, slot, :], s_t,
                                    start=(gi == 0),
                                    stop=(gi == ng - 1
                                          and (SELF_EDGES or ng > 1)),
                                )
                                if gi == 0 and not SELF_EDGES:
                                    for h in range(WPP):
                                        w = p * WPP + h
                                        nc.tensor.matmul(
                                            zps[:, h * 128 : h * 128 + 128],
                                            ps_own2[layer % 2][
                                                :, w * 128 : w * 128 + 128],
                                            ident16[:],
                                            start=False,
                                            stop=(ng == 1 and h == WPP - 1),
                                        )
                                gi += 1

                        # z^T = dinv ⊙ u^T ; -> SBUF f32r (rhs of W matmul)
                        zsT = wpool.tile([128, PAIRW], DT_R, tag="zst")
                        c0 = p * PAIRW
                        nc.vector.tensor_mul(
                            zsT[:], zps[:], dinv_bc[:, c0 : c0 + PAIRW]
                        )

                        hps = phpool.tile([128, PAIRW], F32, tag="h")
                        nc.tensor.matmul(
                            hps[:], w_sb[layer][:], zsT[:],
                            start=True, stop=True,
                        )
                        hT = wpool.tile([128, PAIRW], F32, tag="ht")
                        if layer < 2:
                            nc.scalar.activation(
                                hT[:], hps[:],
                                mybir.ActivationFunctionType.Relu,
                                bias=b_sb[layer][:],
                            )
                        else:
                            nc.scalar.activation(
                                hT[:], hps[:],
                                mybir.ActivationFunctionType.Identity,
                                bias=b_sb[layer][:],
                            )
                        for h in range(WPP):
                            w = p * WPP + h
                            if w >= NW:
                                break
                            tp = ptpool.tile([128, 128], F32, tag="tp")
                            nc.tensor.transpose(
                                tp[:], hT[:, h * 128 : h * 128 + 128],
                                ident_sb[:],
                            )
                            if layer < 2:
                                pst = ps_own2[(layer + 1) % 2][
                                    :, w * 128 : w * 128 + 128]
                                nc.vector.tensor_scalar(
                                    pst, tp[:], dinv_w[:, w : w + 1], None,
                                    op0=mybir.AluOpType.mult,
                                )
                                nc.sync.dma_start(
                                    out=ps_loc[w * 128 : w * 128 + 128, :],
                                    in_=pst,
                                )
                            else:
                                ot = opool.tile([128, 128], F32, tag="oout")
                                nc.scalar.copy(out=ot[:], in_=tp[:])
                                nc.sync.dma_start(
                                    out=out_ext[w * 128 : w * 128 + 128, :],
                                    in_=ot[:],
                                )
                if layer < 2:
                    allgather_ps()

    nc.finalize()
    return nc


_CACHE = {}
TRACE = False          # set by test harness to profile + fill LAST_EXEC_NS
LAST_EXEC_NS = None


def kernel(x, edge_index, W1, b1, W2, b2, W3, b3):
    global LAST_EXEC_NS
    x = np.asarray(x, dtype=np.float32)
    edge_index = np.asarray(edge_index)
    n_nodes = x.shape[0]

    ck = (n_nodes, edge_index.shape[1],
          hash(edge_index.tobytes()))
    if ck in _CACHE:
        meta, nc = _CACHE[ck]
    else:
        meta = _preprocess(edge_index, n_nodes)
        nc = _build(meta)
        _CACHE[ck] = (meta, nc)

    NB, NW, nb_real = meta["NB"], meta["NW"], meta["nb_real"]
    nchunks = meta["nchunks"]

    iota_dt = np.float16 if USE_F16_MSG else np.float32
    iota = np.tile(np.arange(PAIRW, dtype=iota_dt), (128, 1))
    ident = np.eye(128, dtype=np.float32)
    ones1 = np.ones((1, 128), dtype=np.float32)
    ws = [np.asarray(W1, np.float32), np.asarray(W2, np.float32),
          np.asarray(W3, np.float32)]
    bs = [np.asarray(b1, np.float32), np.asarray(b2, np.float32),
          np.asarray(b3, np.float32)]

    gp_node = meta["gp_node"]
    glob_of_local = meta["glob_of_local"]
    x_pad = np.zeros((NB * NCORES, D), dtype=np.float32)
    x_pad[gp_node] = x

    in_maps = []
    for c in range(NCORES):
        xb = np.ascontiguousarray(x_pad[glob_of_local[c]])
        im = {
            "x": xb,
            "degw": meta["deg_w"][c],
            "iota": iota,
            "ident": ident,
            "ones1": ones1,
            "dstw": meta["dstw"][c].astype(
                np.float16 if (USE_F16_MSG and BATCH_ISEQ) else np.float32),
        }
        for l in range(3):
            im[f"w{l}"] = ws[l]
            im[f"b{l}"] = bs[l].reshape(128, 1)
        for k in range(nchunks):
            im[f"idx{k}"] = meta["idx_wrapped"][k][c]
        in_maps.append(im)

    res = bass_utils.run_bass_kernel_spmd(
        nc, in_maps, core_ids=list(range(NCORES)), trace=TRACE
    )
    LAST_EXEC_NS = res.exec_time_ns

    out_pad = np.empty((NB * NCORES, D), dtype=np.float32)
    for c in range(NCORES):
        out_pad[glob_of_local[c]] = res.results[c]["out"]
    return out_pad[gp_node]



# revision 68
# speedup vs baseline: 1.3164x; 1.0611x over previous
"""3-layer GCN on 8 Trainium2 NeuronCores (Bass/Tile SPMD kernel).

Strategy (sharding_hint: shard nodes + edges by destination, replicate
weights, all-gather activations between layers):

  - Nodes are split into 8 contiguous blocks (padded to a multiple of 128
    rows per core).  Core c owns destination block c.
  - Per layer, using linearity of GCN aggregation:
        out_i = [sum_{j->i} dinv_i dinv_j p_j + dinv_i^2 p_i] @ W + b
    with p = previous activations.  We store ps = dinv * p ("scaled"
    activations) so every message (including the self loop, added as an
    explicit edge) has unit coefficient:
        z_i = dinv_i * segment_sum(ps[src])       (over edges + self edges)
        out_i = z_i @ W + b ; next ps = dinv * relu(out)
  - Each core DMA-gathers ps[src] rows (512B each) for its edges from a
    replicated full-activation DRAM buffer, reduces them into per-256-dst
    "window pair" PSUM tiles with one-hot matmuls (one-hots built by DVE
    is_equal against an iota row), applies dinv, multiplies by W (PE),
    bias+ReLU (ACT), rescales, and writes its 1/8 output block.
  - An AllGather (ncfw collective) replicates the per-core ps blocks
    between layers.  Matmul operands use float32r (TF32-like, ~1e-4 rel).

Edges are sorted by (dst core, dst window pair, src chunk); each
(pair, chunk) cell is padded to whole 128-token groups with a group count
equalized across cores so a single SPMD program serves all 8 cores.
Gather indices are int16 (chunk-relative, chunks of <=32768 rows).
"""

import sys

if "/opt/trn_rl_repo" not in sys.path:
    sys.path.insert(0, "/opt/trn_rl_repo")

import numpy as np

import concourse.bacc as bacc
import concourse.mybir as mybir
import concourse.tile as tile
from concourse import bass_utils

F32 = mybir.dt.float32
F32R = mybir.dt.float32r
F16 = mybir.dt.float16
I16 = mybir.dt.int16

NCORES = 8
D = 128
USE_F32R = True      # float32r (TF32-like) matmul operands; False = exact fp32
USE_F16_MSG = True   # fp16 gathered messages + one-hot S (halves gather bytes)
import os as _os

BATCH_ISEQ = _os.environ.get("BATCH_ISEQ", "1") == "1"
SINGLE_PACKET = _os.environ.get("SINGLE_PACKET", "0") == "1"
BALANCE = _os.environ.get("BALANCE", "1") == "1"
SELF_EDGES = _os.environ.get("SELF_EDGES", "0") == "1"
NQ = 4               # SWDGE queues for gather desc-gen parallelism
PAIRW = 256          # dst window width (S matrix / PSUM free size)
WPP = PAIRW // 128   # 128-row windows per dst bin
RP = 2               # dst bins per gather range
NCHUNKS = 4          # gather source chunks = collective quarters


def _assign_balanced(dst, n_nodes, NPAIR):
    """LPT greedy: assign dst rows to (core, pair) bins equalizing total
    token count per bin (in-degree + self loop), capacity PAIRW rows.

    Per-chunk counts then vary only by the multinomial split of each
    bin's sources over the 4 chunks (sigma ~ sqrt(total)/2), vs the
    ~4x larger variance of blocked assignment."""
    import heapq

    NBINS = NCORES * NPAIR
    indeg = np.bincount(dst, minlength=n_nodes).astype(np.int64)
    if SELF_EDGES:
        indeg += 1
    order = np.argsort(-indeg, kind="stable")
    cap = np.full(NBINS, PAIRW, dtype=np.int64)
    assign = np.empty(n_nodes, dtype=np.int64)
    heap = [(0, b) for b in range(NBINS)]
    heapq.heapify(heap)
    for i in order:
        while True:
            load, b = heapq.heappop(heap)
            if cap[b] > 0:
                break
        assign[i] = b
        cap[b] -= 1
        if cap[b] > 0:
            heapq.heappush(heap, (load + int(indeg[i]), b))
    return assign


def _preprocess(edge_index, n_nodes):
    """Host-side integer preprocessing: degrees, edge partition, padding.

    Returns a dict with the static program structure (identical across
    cores) and per-core gather/one-hot metadata arrays.
    """
    src = np.asarray(edge_index[0], dtype=np.int64)
    dst = np.asarray(edge_index[1], dtype=np.int64)

    nb_real = -(-n_nodes // NCORES)              # real rows per core
    NB = -(-nb_real // 128) * 128                # padded rows per core
    NPAD = NB * NCORES
    NW = NB // 128                               # windows per core
    NPAIR = -(-NW // WPP)                        # dst bins per core
    nchunks = NCHUNKS

    deg = np.ones(n_nodes, dtype=np.float64)
    np.add.at(deg, dst, 1.0)                     # bincount, +1 self loop
    deg = deg.astype(np.float32)

    allnodes = np.arange(n_nodes, dtype=np.int64)

    if BALANCE:
        bin_of = _assign_balanced(dst, n_nodes, NPAIR)
        order_r = np.argsort(bin_of, kind="stable")
        poff_node = np.empty(n_nodes, dtype=np.int64)
        starts = np.zeros(NCORES * NPAIR + 1, dtype=np.int64)
        np.cumsum(np.bincount(bin_of, minlength=NCORES * NPAIR),
                  out=starts[1:])
        poff_node[order_r] = np.arange(n_nodes) - starts[bin_of[order_r]]
    else:
        bin_of = ((allnodes // nb_real) * NPAIR
                  + (allnodes % nb_real) // PAIRW)
        poff_node = (allnodes % nb_real) % PAIRW

    # local row -> global padded row (core-major; AllGather concat order)
    CHR = NPAD // nchunks
    csize = [CHR] * nchunks
    chunk_base = np.arange(nchunks + 1, dtype=np.int64) * CHR
    loc_r = np.arange(NB, dtype=np.int64)
    glob_of_local = np.empty((NCORES, NB), dtype=np.int64)
    for c in range(NCORES):
        glob_of_local[c] = c * NB + loc_r
    loc_node = (bin_of % NPAIR) * PAIRW + poff_node
    gp_node = glob_of_local[bin_of // NPAIR, loc_node]

    # self-loop terms are added on-chip (PSUM init transposes), not as
    # edges — unless SELF_EDGES (debug fallback to explicit self tokens)
    if SELF_EDGES:
        s_all = np.concatenate([src, allnodes])
        d_all = np.concatenate([dst, allnodes])
    else:
        s_all = src
        d_all = dst

    core = bin_of[d_all] // NPAIR
    pair = bin_of[d_all] % NPAIR
    poff = poff_node[d_all]                      # offset within bin
    sgp = gp_node[s_all]
    chunk = np.searchsorted(chunk_base, sgp, side="right") - 1
    sidx = (sgp - chunk_base[chunk]).astype(np.int64)

    # sort by (core, pair, chunk), then src within each cell (HBM locality)
    key = ((core * NPAIR) + pair) * nchunks + chunk
    order = np.lexsort((sidx, key))
    key_s = key[order]
    sidx_s = sidx[order]
    poff_s = poff[order]

    ncells = NPAIR * nchunks
    # per-core per-cell counts
    counts = np.zeros((NCORES, ncells), dtype=np.int64)
    uk, uc = np.unique(key_s, return_counts=True)
    counts.reshape(-1)[uk] = uc
    gcell = (-(-counts // 128)).max(axis=0)      # equalized group counts
    gcell = gcell.reshape(NPAIR, nchunks)        # [pair, chunk]

    gtot = int(gcell.sum())
    # stream layout: for p in pairs: for k in chunks: gcell[p,k] groups
    cell_goff = np.zeros((NPAIR, nchunks), dtype=np.int64)
    g = 0
    for p in range(NPAIR):
        for k in range(nchunks):
            cell_goff[p, k] = g
            g += gcell[p, k]

    # per-core padded token arrays in stream order (token-major flat)
    dstw_flat = np.full((NCORES, gtot * 128), -1.0, dtype=np.float32)

    # chunk stream group offsets (within each chunk's gather stream)
    chunk_goff = np.zeros((NPAIR, nchunks), dtype=np.int64)
    acc = np.zeros(nchunks, dtype=np.int64)
    for p in range(NPAIR):
        for k in range(nchunks):
            chunk_goff[p, k] = acc[k]
            acc[k] += gcell[p, k]
    gchunk = acc                                  # groups per chunk stream

    idx_streams = [
        np.zeros((NCORES, int(gchunk[k]) * 128), dtype=np.int16)
        for k in range(nchunks)
    ]

    cell_starts = np.zeros(NCORES * ncells + 1, dtype=np.int64)
    np.cumsum(counts.reshape(-1), out=cell_starts[1:])
    for c in range(NCORES):
        for p in range(NPAIR):
            for k in range(nchunks):
                cell = (c * NPAIR + p) * nchunks + k
                t0, t1 = cell_starts[cell], cell_starts[cell + 1]
                n = t1 - t0
                gk0 = chunk_goff[p, k] * 128
                idx_streams[k][c, gk0 : gk0 + n] = sidx_s[t0:t1]
                g0 = cell_goff[p, k]
                dstw_flat[c, g0 * 128 : g0 * 128 + n] = poff_s[t0:t1]
                # pads keep idx 0 / dstw -1

    # dstw: token t of group g -> [t%128, g]
    dstw = np.ascontiguousarray(
        dstw_flat.reshape(NCORES, gtot, 128).transpose(0, 2, 1)
    )

    # wrap indices: token i -> [i%16, i//16], replicated to 128 partitions
    idx_wrapped = []
    for k in range(nchunks):
        st = idx_streams[k]
        cols = st.shape[1] // 16
        w = st.reshape(NCORES, cols, 16).transpose(0, 2, 1)  # [C,16,cols]
        idx_wrapped.append(np.tile(w, (1, 8, 1)).copy())     # [C,128,cols]

    # degree layouts
    deg_pad = np.ones(NPAD, dtype=np.float32)
    deg_pad[gp_node] = deg
    deg_w = np.empty((NCORES, 128, NW), dtype=np.float32)    # wrapped
    deg_r = np.empty((NCORES, 1, NB), dtype=np.float32)      # row
    for c in range(NCORES):
        blk = deg_pad[glob_of_local[c]]
        deg_w[c] = blk.reshape(NW, 128).T
        deg_r[c, 0] = blk

    return dict(
        NB=NB, NPAD=NPAD, NW=NW, NPAIR=NPAIR, nchunks=nchunks,
        nb_real=nb_real, gcell=gcell, gtot=gtot, gchunk=gchunk,
        cell_goff=cell_goff, chunk_goff=chunk_goff,
        idx_wrapped=idx_wrapped, dstw=dstw, deg_w=deg_w, deg_r=deg_r,
        gp_node=gp_node, glob_of_local=glob_of_local,
        csize=csize,
    )


def _build(meta):
    NB, NPAD, NW, NPAIR = meta["NB"], meta["NPAD"], meta["NW"], meta["NPAIR"]
    nchunks, gcell, gtot = meta["nchunks"], meta["gcell"], meta["gtot"]
    gchunk, cell_goff, chunk_goff = (
        meta["gchunk"], meta["cell_goff"], meta["chunk_goff"],
    )

    DT_R = F32R if USE_F32R else F32
    DT_M = F16 if USE_F16_MSG else DT_R     # message/S dtype

    NGMAX = int(gcell.sum(axis=1).max())

    nc = bacc.Bacc(None, target_bir_lowering=False, num_devices=NCORES,
                   num_swdge_queues=NQ)

    x_ext = nc.dram_tensor("x", [NB, D], F32, kind="ExternalInput")
    degw_ext = nc.dram_tensor("degw", [128, NW], F32, kind="ExternalInput")
    iota_ext = nc.dram_tensor("iota", [128, PAIRW],
                              F16 if USE_F16_MSG else F32,
                              kind="ExternalInput")
    ident_ext = nc.dram_tensor("ident", [128, 128], F32, kind="ExternalInput")
    ones_ext = nc.dram_tensor("ones1", [1, 128], F32, kind="ExternalInput")
    w_ext = [
        nc.dram_tensor(f"w{l}", [D, D], F32, kind="ExternalInput")
        for l in range(3)
    ]
    b_ext = [
        nc.dram_tensor(f"b{l}", [128, 1], F32, kind="ExternalInput")
        for l in range(3)
    ]
    idx_ext = [
        nc.dram_tensor(f"idx{k}", [128, int(gchunk[k]) * 8], I16,
                       kind="ExternalInput")
        for k in range(nchunks)
    ]
    dstw_ext = nc.dram_tensor("dstw", [128, gtot],
                              F16 if (USE_F16_MSG and BATCH_ISEQ) else F32,
                              kind="ExternalInput")
    out_ext = nc.dram_tensor("out", [NB, D], F32, kind="ExternalOutput")

    csize = meta["csize"]
    cb = np.concatenate([[0], np.cumsum(csize)]).astype(np.int64)
    ps_loc = nc.dram_tensor("ps_loc", [NB, D], DT_M)
    ps_full_all = nc.dram_tensor("ps_full", [NPAD, D], DT_M,
                                 addr_space="Shared")
    ps_full = [ps_full_all[int(cb[k]) : int(cb[k + 1]), :]
               for k in range(nchunks)]

    def allgather_ps():
        nc.gpsimd.collective_compute(
            "AllGather", mybir.AluOpType.bypass,
            replica_groups=[list(range(NCORES))],
            ins=[ps_loc.ap().opt()], outs=[ps_full_all.ap().opt()],
        )

    QROT = [0]

    # gather ranges: RP pairs each
    ranges = [list(range(r, min(r + RP, NPAIR))) for r in range(0, NPAIR, RP)]

    with tile.TileContext(nc) as tc:
        with (
            tc.tile_pool(name="const", bufs=1) as cpool,
            tc.tile_pool(name="msg", bufs=3) as mpool,
            tc.tile_pool(name="idxp", bufs=2) as ipool,
            tc.tile_pool(name="sbld", bufs=2) as spool,
            tc.tile_pool(name="work", bufs=3) as wpool,
            tc.tile_pool(name="outp", bufs=4) as opool,
            tc.tile_pool(name="pz", bufs=3, space="PSUM") as pzpool,
            tc.tile_pool(name="pt", bufs=2, space="PSUM") as ptpool,
            tc.tile_pool(name="ph", bufs=3, space="PSUM") as phpool,
        ):
            # ---- constants ----
            iota_sb = cpool.tile([128, PAIRW], F16 if USE_F16_MSG else F32)
            nc.sync.dma_start(out=iota_sb[:], in_=iota_ext[:, :])
            ident_sb = cpool.tile([128, 128], F32)
            nc.sync.dma_start(out=ident_sb[:], in_=ident_ext[:, :])
            ident16 = cpool.tile([128, 128], DT_M, tag="ident16")
            nc.vector.tensor_copy(ident16[:], ident_sb[:])
            ones_sb = cpool.tile([1, 128], F32)
            nc.sync.dma_start(out=ones_sb[:], in_=ones_ext[:, :])
            w_sb = []
            for l in range(3):
                wt = cpool.tile([D, D], F32, tag=f"wraw{l}")
                nc.sync.dma_start(out=wt[:], in_=w_ext[l][:, :])
                if USE_F32R:
                    wr = cpool.tile([D, D], F32R, tag=f"w{l}")
                    nc.vector.tensor_copy(wr[:], wt[:])
                    w_sb.append(wr)
                else:
                    w_sb.append(wt)
            b_sb = []
            for l in range(3):
                bt = cpool.tile([128, 1], F32, tag=f"b{l}")
                nc.sync.dma_start(out=bt[:], in_=b_ext[l][:, :])
                b_sb.append(bt)
            dstw_sb = cpool.tile(
                [128, gtot], F16 if (USE_F16_MSG and BATCH_ISEQ) else F32)
            nc.sync.dma_start(out=dstw_sb[:], in_=dstw_ext[:, :])

            # ---- dinv (wrapped + broadcast along free dim) ----
            degw_sb = cpool.tile([128, NW], F32)
            nc.sync.dma_start(out=degw_sb[:], in_=degw_ext[:, :])
            rcpw = cpool.tile([128, NW], F32)
            nc.vector.reciprocal(rcpw[:], degw_sb[:])
            dinv_w = cpool.tile([128, NW], F32)
            nc.scalar.activation(dinv_w[:], rcpw[:],
                                 mybir.ActivationFunctionType.Sqrt)

            # dinv_bc[p, d] = dinv[d]: per window, move the dinv column to a
            # partition-0 row (matmul vs identity), then K=1 ones-broadcast.
            dinv_bc = cpool.tile([128, NB], DT_M)
            for w in range(NW):
                pr = ptpool.tile([128, 128], F32, tag="tp")
                nc.tensor.matmul(pr[0:1, :], dinv_w[:, w : w + 1],
                                 ident_sb[:], start=True, stop=True)
                row_sb = wpool.tile([1, 128], F32, tag="drow")
                nc.scalar.copy(out=row_sb[:], in_=pr[0:1, :])
                pb = ptpool.tile([128, 128], F32, tag="tp")
                nc.tensor.matmul(pb[:], ones_sb[:], row_sb[:],
                                 start=True, stop=True)
                nc.scalar.copy(
                    out=dinv_bc[:, w * 128 : w * 128 + 128], in_=pb[:]
                )

            # ---- own-block ps mirror (for self-loop PSUM init) ----
            # double-buffered across layers: layer L reads ps in buffer
            # L %% 2 while its epilogue writes the next ps into (L+1) %% 2 —
            # no write-after-read window on a live buffer.
            ps_own_a = cpool.tile([128, NB], DT_M, tag="psown0")
            ps_own_b = cpool.tile([128, NB], DT_M, tag="psown1")
            ps_own2 = [ps_own_a, ps_own_b]
            ps_own = ps_own_a

            # ---- prologue: ps0 = dinv * x ----
            for w in range(NW):
                xt = wpool.tile([128, 128], F32, tag="xin")
                nc.sync.dma_start(out=xt[:], in_=x_ext[w * 128 : w * 128 + 128, :])
                xs = ps_own[:, w * 128 : w * 128 + 128]
                nc.vector.tensor_scalar(
                    xs, xt[:], dinv_w[:, w : w + 1], None,
                    op0=mybir.AluOpType.mult,
                )
                nc.sync.dma_start(
                    out=ps_loc[w * 128 : w * 128 + 128, :], in_=xs
                )
            allgather_ps()

            # ---- layers ----
            for layer in range(3):
                for rng_pairs in ranges:
                    # gather all chunks for this range
                    mtiles = {}
                    for k in range(nchunks):
                        g_rk = int(sum(gcell[p, k] for p in rng_pairs))
                        if g_rk == 0:
                            continue
                        g0 = int(chunk_goff[rng_pairs[0], k])
                        ni = g_rk * 128
                        it = ipool.tile([128, ni // 16], I16, tag=f"i{k}")
                        nc.sync.dma_start(
                            out=it[:],
                            in_=idx_ext[k][:, g0 * 8 : g0 * 8 + ni // 16],
                        )
                        mt = mpool.tile([128, g_rk, 128], DT_M, tag=f"m{k}")
                        nc.gpsimd.dma_gather(
                            mt[:],
                            ps_full[k][:, :],
                            it[:],
                            ni, ni, D,
                            single_packet=SINGLE_PACKET,
                            queue_num=QROT[0] % NQ,
                        )
                        QROT[0] += 1
                        mtiles[k] = (mt, g0)

                    for p in rng_pairs:
                        # segment-sum into PSUM [feat, PAIRW]; the self-loop
                        # term psT (unit coefficient) accumulates via PE
                        # identity matmuls after the group opens.
                        zps = pzpool.tile([128, PAIRW], F32, tag="zacc")
                        ng = int(sum(gcell[p, k] for k in range(nchunks)))
                        assert ng > 0
                        # batched one-hot build: ONE DVE is_equal for all
                        # of this pair's groups, via stride-0 broadcast APs.
                        # s_big[:, g, d] = (iota[:, d] == dstw[:, g0r + g])
                        g0r = int(cell_goff[p, 0])
                        if BATCH_ISEQ:
                            s_big = spool.tile([128, NGMAX, PAIRW], DT_M,
                                               tag="sbig")
                            in0 = iota_sb[:].unsqueeze(1).to_broadcast(
                                [128, ng, PAIRW])
                            in1 = dstw_sb[:, g0r : g0r + ng].unsqueeze(
                                2).to_broadcast([128, ng, PAIRW])
                            nc.vector.tensor_tensor(
                                s_big[:, 0:ng, :], in0, in1,
                                op=mybir.AluOpType.is_equal,
                            )
                        gi = 0
                        for k in range(nchunks):
                            for j in range(int(gcell[p, k])):
                                mt, g0 = mtiles[k]
                                slot = int(chunk_goff[p, k]) - g0 + j
                                gcol = int(cell_goff[p, k]) + j
                                if BATCH_ISEQ:
                                    s_t = s_big[:, gcol - g0r, :]
                                else:
                                    st = spool.tile([128, PAIRW], DT_M,
                                                    tag="s")
                                    nc.vector.tensor_scalar(
                                        st[:], iota_sb[:],
                                        dstw_sb[:, gcol : gcol + 1], None,
                                        op0=mybir.AluOpType.is_equal,
                                    )
                                    s_t = st[:]
                                # first matmul opens the accumulation group
                                # full-width (start=True); the self-term
                                # init matmuls then ACCUMULATE psT of the
                                # pair's two windows (safe regardless of
                                # whether start clears per-address or
                                # whole-bank has_written state).
                                nc.tensor.matmul(
                                    zps[:], mt[:, slot, :], s_t,
                                    start=(gi == 0),
                                    stop=(gi == ng - 1
                                          and (SELF_EDGES or ng > 1)),
                                )
                                if gi == 0 and not SELF_EDGES:
                                    for h in range(WPP):
                                        w = p * WPP + h
                                        nc.tensor.matmul(
                                            zps[:, h * 128 : h * 128 + 128],
                                            ps_own2[layer % 2][
                                                :, w * 128 : w * 128 + 128],
                                            ident16[:],
                                            start=False,
                                            stop=(ng == 1 and h == WPP - 1),
                                        )
                                gi += 1

                        # z^T = dinv ⊙ u^T ; -> SBUF f32r (rhs of W matmul)
                        zsT = wpool.tile([128, PAIRW], DT_R, tag="zst")
                        c0 = p * PAIRW
                        nc.vector.tensor_mul(
                            zsT[:], zps[:], dinv_bc[:, c0 : c0 + PAIRW]
                        )

                        hps = phpool.tile([128, PAIRW], F32, tag="h")
                        nc.tensor.matmul(
                            hps[:], w_sb[layer][:], zsT[:],
                            start=True, stop=True,
                        )
                        hT = wpool.tile([128, PAIRW], F32, tag="ht")
                        if layer < 2:
                            nc.scalar.activation(
                                hT[:], hps[:],
                                mybir.ActivationFunctionType.Relu,
                                bias=b_sb[layer][:],
                            )
                        else:
                            nc.scalar.activation(
                                hT[:], hps[:],
                                mybir.ActivationFunctionType.Identity,
                                bias=b_sb[layer][:],
                            )
                        for h in range(WPP):
                            w = p * WPP + h
                            if w >= NW:
                                break
                            tp = ptpool.tile([128, 128], F32, tag="tp")
                            nc.tensor.transpose(
                                tp[:], hT[:, h * 128 : h * 128 + 128],
                                ident_sb[:],
                            )
                            if layer < 2:
                                pst = ps_own2[(layer + 1) % 2][
                                    :, w * 128 : w * 128 + 128]
                                nc.vector.tensor_scalar(
                                    pst, tp[:], dinv_w[:, w : w + 1], None,
                                    op0=mybir.AluOpType.mult,
                                )
                                nc.sync.dma_start(
                                    out=ps_loc[w * 128 : w * 128 + 128, :],
                                    in_=pst,
                                )
                            else:
                                ot = opool.tile([128, 128], F32, tag="oout")
                                nc.scalar.copy(out=ot[:], in_=tp[:])
                                nc.sync.dma_start(
                                    out=out_ext[w * 128 : w * 128 + 128, :],
                                    in_=ot[:],
                                )
                if layer < 2:
                    allgather_ps()

    nc.finalize()
    return nc


_CACHE = {}
TRACE = False          # set by test harness to profile + fill LAST_EXEC_NS
LAST_EXEC_NS = None


def kernel(x, edge_index, W1, b1, W2, b2, W3, b3):
    global LAST_EXEC_NS
    x = np.asarray(x, dtype=np.float32)
    edge_index = np.asarray(edge_index)
    n_nodes = x.shape[0]

    ck = (n_nodes, edge_index.shape[1],
          hash(edge_index.tobytes()))
    if ck in _CACHE:
        meta, nc = _CACHE[ck]
    else:
        meta = _preprocess(edge_index, n_nodes)
        nc = _build(meta)
        _CACHE[ck] = (meta, nc)

    NB, NW, nb_real = meta["NB"], meta["NW"], meta["nb_real"]
    nchunks = meta["nchunks"]

    iota_dt = np.float16 if USE_F16_MSG else np.float32
    iota = np.tile(np.arange(PAIRW, dtype=iota_dt), (128, 1))
    ident = np.eye(128, dtype=np.float32)
    ones1 = np.ones((1, 128), dtype=np.float32)
    ws = [np.asarray(W1, np.float32), np.asarray(W2, np.float32),
          np.asarray(W3, np.float32)]
    bs = [np.asarray(b1, np.float32), np.asarray(b2, np.float32),
          np.asarray(b3, np.float32)]

    gp_node = meta["gp_node"]
    glob_of_local = meta["glob_of_local"]
    x_pad = np.zeros((NB * NCORES, D), dtype=np.float32)
    x_pad[gp_node] = x

    in_maps = []
    for c in range(NCORES):
        xb = np.ascontiguousarray(x_pad[glob_of_local[c]])
        im = {
            "x": xb,
            "degw": meta["deg_w"][c],
            "iota": iota,
            "ident": ident,
            "ones1": ones1,
            "dstw": meta["dstw"][c].astype(
                np.float16 if (USE_F16_MSG and BATCH_ISEQ) else np.float32),
        }
        for l in range(3):
            im[f"w{l}"] = ws[l]
            im[f"b{l}"] = bs[l].reshape(128, 1)
        for k in range(nchunks):
            im[f"idx{k}"] = meta["idx_wrapped"][k][c]
        in_maps.append(im)

    res = bass_utils.run_bass_kernel_spmd(
        nc, in_maps, core_ids=list(range(NCORES)), trace=TRACE
    )
    LAST_EXEC_NS = res.exec_time_ns

    out_pad = np.empty((NB * NCORES, D), dtype=np.float32)
    for c in range(NCORES):
        out_pad[glob_of_local[c]] = res.results[c]["out"]
    return out_pad[gp_node]

